# revision 36
# baseline (speedup 1.0000x reference)
"""AlphaQubit-like decoder on 8 TRN2 NeuronCores (Bass/Tile).

Sharding: data-parallel over batch (16 per core), zero collectives.
Activations are feature-major [d (partitions), (b, l) (free)] so every
matmul contracts on partitions with weights stationary and every bias is
per-partition. bf16 matmul operands; f32 residual stream + PSUM.

Structure (2.21ms -> 1.46ms vs the per-(b,h) baseline):
  - bq folded into Q at QKV PSUM evacuation (per-partition bias), bk
    dropped (softmax-invariant), bv folded via bo_eff = bo + Wo^T bv;
    the (X+S)/sqrt2 prescale is applied host-side.
  - Pairwise-bias projection Bp computed with a block-diagonal lhs over
    m-chunks of 8 so the output lands m-major ([*(h,mj), (b,l)]), then
    stored to DRAM as [NH, m, (b,l)] with fat contiguous rows; attention
    loads one [120, 1920] head tile per (l,h) with 3.8KB descriptors
    (the v1 thin-row transpose loads burned ~700us of DMA-queue time).
  - Attention batched per (l,h) in a 3-stage head pipeline: S^T in
    [120,480] PSUM quarters (+Bp via identity matmul), one exp per
    quarter; row-sums computed TRANSPOSED ([120(l),16(b)] via E-as-lhs
    N=1 matmuls) so the DVE reciprocal (6.3ns/free-elem!) runs on 16
    free elems instead of 46080; orientation restored by an
    identity-rhs transpose matmul + delta-broadcast matmuls; one DVE
    multiply per quarter normalizes A@V.
  - LayerNorm rs = exp(-0.5*ln(var+eps)) on the Activation engine (ln
    and exp share one act table with attention's exp; no DVE
    reciprocal); squares on DVE in bf16; mean/rs broadcast by K=1
    matmuls; chunk-staggered emission keeps the scalar rsqrt tail off
    the PE queue's head.
  - FFN gate: (fc1g + b) * gelu(fc1a + b) via one scalar_tensor_tensor.
  - Conv LN writer uses 3D-AP activations; conv matmuls in 3-batch
    N=432 chunks; scatter/gather and pad-zeroing split across engines
    and hoisted off the conv-entry critical path.
  - Bulk DMAs rotate over gpsimd/scalar queues; sync is reserved for
    bias-tile and Bp head loads so they never queue behind stores.
"""

import contextlib
import itertools
import math

import numpy as np
import ml_dtypes

import concourse.bass as bass
import concourse.mybir as mybir
import concourse.tile as tile
from concourse.bass_utils import run_bass_kernel_spmd

F32 = mybir.dt.float32
BF16 = mybir.dt.bfloat16
AX = mybir.AluOpType
AF = mybir.ActivationFunctionType

B, L, DD = 128, 120, 256
DA, DM, H, DB = 64, 64, 8, 16
NL, LC = 3, 3
DILS = (1, 2, 4)
G = 12
NCORES = 8
BL = B // NCORES            # 16 batches per core
R = BL * L                  # 1920 seq cols per core
CELLS = G * G               # 144
RG = BL * CELLS             # 2304 grid cols per core
NH = NL * H                 # 24 (layer,head) rows
SP = 20                     # padded grid side (pad=4 fits dilation<=4)
PADC = SP * SP
CHUNK = 480                 # seq chunk (4 batches)
NCH = R // CHUNK            # 4 seq chunks
GCH = 288                   # grid chunk (2 batches)
MC = 15                     # m-chunks of 8 in the Bp projection
EPS = 1e-5
SCALE = 1.0 / math.sqrt(DA)


def _legalize_sync(nc):
    """This walrus build caps per-instruction semaphore waits (drain: 0,
    matmul/ldweights: 1, others: 2); move excess waits onto NoOps."""
    caps = {"InstDrain": 0}
    for f in nc.m.functions:
        for bb in f.blocks:
            new_insts = []
            for inst in bb.instructions:
                si = getattr(inst, "sync_info", None)
                cap = caps.get(type(inst).__name__, 1)
                if si is not None and si.on_wait and len(si.on_wait) > cap:
                    waits = list(si.on_wait)
                    keep = waits[len(waits) - cap:] if cap else []
                    for i, w in enumerate(waits[: len(waits) - cap]):
                        new_insts.append(mybir.InstNoOp(
                            name=inst.name + f"-ws{i}", engine=inst.engine,
                            ins=[], outs=[],
                            sync_info=mybir.SyncInfo(on_wait=[w],
                                                     on_update=[])))
                    si.on_wait = keep
                new_insts.append(inst)
            bb.instructions = new_insts


def build_graph(tok2grid, debug_taps=()):
    dbg = set(debug_taps)
    nc = bass.Bass()

    xT = nc.dram_tensor("xT", [DD, R], F32, kind="ExternalInput")
    sT = nc.dram_tensor("sT", [DD, R], F32, kind="ExternalInput")
    biasT2 = nc.dram_tensor("biasT2", [MC, 128, R], BF16,
                            kind="ExternalInput")
    wbblk = nc.dram_tensor("wbblk", [128, NL, 64], BF16,
                           kind="ExternalInput")
    wqkv = nc.dram_tensor("wqkv", [NL, DD, 1536], BF16, kind="ExternalInput")
    wo = nc.dram_tensor("wo", [NL, H * DM, DD], BF16, kind="ExternalInput")
    fc1w = nc.dram_tensor("fc1w", [NL, DD, 1024], BF16, kind="ExternalInput")
    fc2w = nc.dram_tensor("fc2w", [NL, 512, DD], BF16, kind="ExternalInput")
    cww = nc.dram_tensor("cww", [LC, 2, 128, 9 * DD], BF16,
                         kind="ExternalInput")
    lngP = nc.dram_tensor("lngP", [128, 9, 2], F32, kind="ExternalInput")
    lnbP = nc.dram_tensor("lnbP", [128, 9, 2], F32, kind="ExternalInput")
    boP = nc.dram_tensor("boP", [128, NL, 2], F32, kind="ExternalInput")
    b1P = nc.dram_tensor("b1P", [128, NL, 8], F32, kind="ExternalInput")
    b2P = nc.dram_tensor("b2P", [128, NL, 2], F32, kind="ExternalInput")
    cbP = nc.dram_tensor("cbP", [128, LC, 2], F32, kind="ExternalInput")
    pP = nc.dram_tensor("pP", [128, 2], F32, kind="ExternalInput")
    bqP = nc.dram_tensor("bqP", [128, NL, 4], F32, kind="ExternalInput")
    bvP = nc.dram_tensor("bvP", [128, NL, 4], BF16, kind="ExternalInput")
    eyeD = nc.dram_tensor("eyeD", [128, 128], BF16, kind="ExternalInput")
    obtD = nc.dram_tensor("obtD", [16, 1024], BF16, kind="ExternalInput")
    outD = nc.dram_tensor("out", [DD, R], F32, kind="ExternalOutput")

    dbg_outs = {}

    def dbg_tensor(name, shape):
        dbg_outs[name] = nc.dram_tensor(name, shape, F32,
                                        kind="ExternalOutput")
        return dbg_outs[name]

    # token<->grid runs (host-known permutation baked into APs)
    t2g = [int(v) for v in tok2grid]
    assert len(set(t2g)) == L and all(0 <= v < CELLS for v in t2g)
    runs = []
    s = 0
    for i in range(1, L + 1):
        if i == L or t2g[i] != t2g[i - 1] + 1:
            runs.append((s, t2g[s], i - s))
            s = i
    unmapped = sorted(set(range(CELLS)) - set(t2g))
    uruns = []
    if unmapped:
        s = 0
        for i in range(1, len(unmapped) + 1):
            if i == len(unmapped) or unmapped[i] != unmapped[i - 1] + 1:
                uruns.append((unmapped[s], i - s))
                s = i

    uid = [0]

    def pool(lctx, name, bufs, space="SBUF"):
        uid[0] += 1
        return lctx.enter_context(
            tc.tile_pool(name=f"{name}_{uid[0]}", bufs=bufs, space=space))

    # rotate bulk DMAs across engine queues
    qrr = itertools.cycle(["gpsimd", "scalar"])

    def dma(out, in_):
        getattr(nc, next(qrr)).dma_start(out=out, in_=in_)

    with tile.TileContext(nc) as tc, contextlib.ExitStack() as ctx:
        wp = ctx.enter_context(tc.tile_pool(name="weights", bufs=1))
        xp = ctx.enter_context(tc.tile_pool(name="xres", bufs=2))
        dram = ctx.enter_context(tc.tile_pool(name="dram", bufs=1,
                                              space="DRAM"))

        # ---------------- weights/constants to SBUF
        w_qkv = [[wp.tile([128, 1536], BF16, tag=f"wqkv{l}{k}",
                          name=f"wqkv{l}{k}") for k in range(2)]
                 for l in range(NL)]
        w_o = [[wp.tile([128, DD], BF16, tag=f"wo{l}{k}", name=f"wo{l}{k}")
                for k in range(4)] for l in range(NL)]
        w_f1 = [[wp.tile([128, 1024], BF16, tag=f"f1{l}{k}", name=f"f1{l}{k}")
                 for k in range(2)] for l in range(NL)]
        w_f2 = [[wp.tile([128, DD], BF16, tag=f"f2{l}{k}", name=f"f2{l}{k}")
                 for k in range(4)] for l in range(NL)]
        w_cv = [[wp.tile([128, 9 * DD], BF16, tag=f"cv{c}{k}",
                         name=f"cv{c}{k}") for k in range(2)]
                for c in range(LC)]
        for k in range(2):
            dma(w_qkv[0][k][:], wqkv[0, 128 * k:128 * (k + 1), :])
        for l in range(NL):
            for k in range(4):
                dma(w_o[l][k][:], wo[l, 128 * k:128 * (k + 1), :])

        def load_late_weights():
            for l in range(NL):
                for k in range(2):
                    if l > 0:
                        dma(w_qkv[l][k][:],
                            wqkv[l, 128 * k:128 * (k + 1), :])
                    dma(w_f1[l][k][:], fc1w[l, 128 * k:128 * (k + 1), :])
                for k in range(4):
                    dma(w_f2[l][k][:], fc2w[l, 128 * k:128 * (k + 1), :])
            for c in range(LC):
                for k in range(2):
                    dma(w_cv[c][k][:], cww[c, k, :, :])

        lng = wp.tile([128, 9, 2], F32)
        dma(lng[:], lngP[:])
        lnb = wp.tile([128, 9, 2], F32)
        dma(lnb[:], lnbP[:])
        bo_s = wp.tile([128, NL, 2], F32)
        dma(bo_s[:], boP[:])
        b1_s = wp.tile([128, NL, 8], F32)
        dma(b1_s[:], b1P[:])
        b2_s = wp.tile([128, NL, 2], F32)
        dma(b2_s[:], b2P[:])
        cb_s = wp.tile([128, LC, 2], F32)
        dma(cb_s[:], cbP[:])
        p_s = wp.tile([128, 2], F32)
        dma(p_s[:], pP[:])
        bq_s = wp.tile([128, NL, 4], F32)
        dma(bq_s[:], bqP[:])
        bv_s = wp.tile([128, NL, 4], BF16)
        dma(bv_s[:], bvP[:])
        eye_s = wp.tile([128, 128], BF16)
        dma(eye_s[:], eyeD[:])
        obt_s = wp.tile([16, 1024], BF16)
        dma(obt_s[:], obtD[:])
        wbb_s = wp.tile([128, NL, 64], BF16)
        dma(wbb_s[:], wbblk[:])

        ones_rowb = wp.tile([1, 128], BF16)
        nc.vector.memset(ones_rowb[:], 1.0)
        invd_col = wp.tile([128, 1], BF16)
        nc.vector.memset(invd_col[:], 1.0 / DD)
        onec = wp.tile([128, 1], BF16)
        nc.vector.memset(onec[:], 1.0)
        eps_col = wp.tile([128, 1], F32)
        nc.vector.memset(eps_col[:], EPS)
        zero_col = wp.tile([128, 1], F32)
        nc.vector.memset(zero_col[:], 0.0)

        # fold bv into bo: bo_eff = bo + Wo^T bv
        bo_eff = wp.tile([128, NL, 2], F32)
        with tc.tile_pool(name="prep_ps", bufs=2, space="PSUM") as pps:
            for l in range(NL):
                d_ps = pps.tile([128, 2], F32, tag="dps", name="dps")
                for ot in range(2):
                    for k in range(4):
                        nc.tensor.matmul(
                            d_ps[:, ot:ot + 1],
                            w_o[l][k][:, 128 * ot:128 * (ot + 1)],
                            bv_s[:, l, k:k + 1],
                            start=(k == 0), stop=(k == 3))
                nc.vector.tensor_add(out=bo_eff[:, l], in0=d_ps[:],
                                     in1=bo_s[:, l])

        # ---------------- residual init: x = X + S (host pre-scaled 1/sqrt2)
        X = [[xp.tile([128, R], F32, tag=f"X{t}", name=f"X{t}")
              for t in range(2)]]
        with contextlib.ExitStack() as lctx:
            ipool = pool(lctx, "init", 2)
            for t in range(2):
                xi = ipool.tile([128, R], F32, tag="xi", name="xi")
                si = ipool.tile([128, R], F32, tag="si", name="si")
                nc.sync.dma_start(out=xi[:], in_=xT[128 * t:128 * (t + 1), :])
                nc.sync.dma_start(out=si[:], in_=sT[128 * t:128 * (t + 1), :])
                nc.vector.tensor_add(out=X[0][t][:], in0=xi[:], in1=si[:])
        if "x0" in dbg:
            d = dbg_tensor("d_x0", [DD, R])
            for t in range(2):
                nc.sync.dma_start(out=d[128 * t:128 * (t + 1), :],
                                  in_=X[0][t][:])

        # ---------------- layernorm (feature-major, chunk-staggered)
        def layernorm(lctx, li, src, n_chunks, ccols, writer):
            """src: 2 f32 tiles [128, n_chunks*ccols]. writer(t, c, xc, rsb)
            must emit (xc * g) * rs + b into its target; xc f32 SBUF
            [128, ccols], rsb f32 PSUM [128, ccols] broadcast of rs.
            Stats matmuls of chunk c+1 are emitted before the broadcast of
            chunk c so the PE queue never stalls on the scalar rsqrt tail."""
            sps = pool(lctx, "ln_st", 2, "PSUM")
            bps = pool(lctx, "ln_bc", 2, "PSUM")
            sp = pool(lctx, "ln_sb", 2)

            def stats(c):
                cs = slice(c * ccols, (c + 1) * ccols)
                xb = [sp.tile([128, 2, ccols], BF16, tag=f"xb{t}",
                              name=f"xb{t}") for t in range(2)]
                for t in range(2):
                    nc.gpsimd.dma_start(out=xb[t][:, 0], in_=src[t][:, cs])
                for t in range(2):   # squares on DVE (bf16 fast mode)
                    nc.vector.tensor_mul(out=xb[t][:, 1], in0=xb[t][:, 0],
                                         in1=xb[t][:, 0])
                st = sps.tile([1, 2, 512], F32, tag="st", name="st")
                for t in range(2):
                    nc.tensor.matmul(st[:, 0, :ccols], invd_col[:],
                                     xb[t][:, 0], start=(t == 0),
                                     stop=(t == 1))
                for t in range(2):
                    nc.tensor.matmul(st[:, 1, :ccols], invd_col[:],
                                     xb[t][:, 1], start=(t == 0),
                                     stop=(t == 1))
                return st

            def tail(c, st):
                cs = slice(c * ccols, (c + 1) * ccols)
                mrs = sp.tile([1, 2, ccols], BF16, tag="mrs", name="mrs")
                nc.scalar.copy(mrs[:, 0], st[:, 0, :ccols])
                sqm = sp.tile([1, ccols], F32, tag="sqm", name="sqm")
                nc.scalar.square(sqm[:], mrs[:, 0])
                lnv = sp.tile([1, ccols], F32, tag="lnv", name="lnv")
                nc.vector.tensor_sub(out=lnv[:], in0=st[:, 1, :ccols],
                                     in1=sqm[:])
                # rs = (var+eps)^-1/2 = exp(-0.5*ln(var+eps))
                nc.scalar.activation(lnv[:], lnv[:], AF.Ln,
                                     bias=eps_col[0:1, :], scale=1.0)
                nc.scalar.activation(mrs[:, 1], lnv[:], AF.Exp,
                                     bias=zero_col[0:1, :], scale=-0.5)
                mb = bps.tile([128, ccols], F32, tag="mb", name="mb")
                nc.tensor.matmul(mb[:], ones_rowb[:], mrs[:, 0],
                                 start=True, stop=True)
                rsb = bps.tile([128, ccols], F32, tag="rsb", name="rsb")
                nc.tensor.matmul(rsb[:], ones_rowb[:], mrs[:, 1],
                                 start=True, stop=True)
                for t in range(2):
                    xc = sp.tile([128, ccols], F32, tag=f"xc{t}",
                                 name=f"xc{t}")
                    nc.vector.tensor_sub(out=xc[:], in0=src[t][:, cs],
                                         in1=mb[:])
                    writer(t, c, xc, rsb)

            prev = None
            for c in range(n_chunks):
                st = stats(c)
                if prev is not None:
                    tail(*prev)
                prev = (c, st)
            tail(*prev)

        # ---------------- Bp projection: m-major blocks -> DRAM
        BpTd = dram.tile([NH, 120, R], BF16)

        def emit_bp_projection():
            with contextlib.ExitStack() as lctx:
                btp = pool(lctx, "bp_in", 3)
                bpp = pool(lctx, "bp_ps", 8, "PSUM")
                bst = pool(lctx, "bp_st", 3)
                for mc in range(MC):
                    bt = btp.tile([128, R], BF16, tag="bt", name="bt")
                    nc.sync.dma_start(out=bt[:], in_=biasT2[mc])
                    for l in range(NL):
                        stage = bst.tile([64, R], BF16, tag="bstg",
                                         name="bstg")
                        for q in range(NCH):
                            qs = slice(q * CHUNK, (q + 1) * CHUNK)
                            ps = bpp.tile([64, CHUNK], F32, tag="bpp",
                                          name="bpp")
                            nc.tensor.matmul(ps[:], wbb_s[:, l, :],
                                             bt[:, qs],
                                             start=True, stop=True)
                            if (mc + l + q) % 2 == 0:
                                nc.scalar.copy(stage[:, qs], ps[:])
                            else:
                                nc.vector.tensor_copy(out=stage[:, qs],
                                                      in_=ps[:])
                        for h in range(H):
                            dma(BpTd[8 * l + h, 8 * mc:8 * (mc + 1), :],
                                stage[8 * h:8 * (h + 1), :])

        load_late_weights()

        if dbg:   # debug taps need the projection data eagerly
            emit_bp_projection()
        if "bp" in dbg:
            d_bp = nc.dram_tensor("d_bp", [NH, 120, R], BF16,
                                  kind="ExternalOutput")
            dbg_outs["d_bp"] = d_bp
            nc.sync.dma_start(out=d_bp[:], in_=BpTd[:])

        # ---------------- layers
        for l in range(NL):
            Xc = X[l]
            with contextlib.ExitStack() as lp:
                op_ = pool(lp, f"op{l}", 1)    # O^T: lives till Wo
                O = [op_.tile([128, R], BF16, tag=f"o{ot}", name=f"o{ot}")
                     for ot in range(4)]
                qa = contextlib.ExitStack()
                lp.enter_context(qa)
                qv = pool(qa, f"qv{l}", 1)   # qk/V live till attn end
                qk = [qv.tile([128, R], BF16, tag=f"qk{ot}",
                              name=f"qk{ot}") for ot in range(8)]
                V = [qv.tile([120, 512], BF16, tag=f"v{b}",
                             name=f"v{b}") for b in range(BL)]
                # ---- LN1 + QKV (Xn scoped)
                with contextlib.ExitStack() as ph:
                    xnp = pool(ph, f"xn{l}", 1)
                    Xn = [xnp.tile([128, R], BF16, tag=f"xn{t}",
                                   name=f"xn{t}") for t in range(2)]

                    def w_ln1(t, c, xc, rsb, l=l, Xn=Xn):
                        tgt = Xn[t][:, c * CHUNK:(c + 1) * CHUNK]
                        nc.vector.tensor_mul(out=xc[:], in0=xc[:],
                                             in1=rsb[:])
                        nc.vector.tensor_scalar(
                            out=tgt, in0=xc[:],
                            scalar1=lng[:, 2 * l, t:t + 1],
                            scalar2=lnb[:, 2 * l, t:t + 1],
                            op0=AX.mult, op1=AX.add)

                    with contextlib.ExitStack() as lnx:
                        layernorm(lnx, 2 * l, Xc, NCH, CHUNK, w_ln1)
                    if l == 0 and "ln1" in dbg:
                        d = dbg_tensor("d_ln1", [DD, R])
                        for t in range(2):
                            nc.gpsimd.dma_start(
                                out=d[128 * t:128 * (t + 1), :],
                                in_=Xn[t][:])

                    qps = pool(ph, f"qkv_ps{l}", 2, "PSUM")
                    for c in range(NCH):
                        cs = slice(c * CHUNK, (c + 1) * CHUNK)
                        for ot in range(8):
                            ps = qps.tile([128, CHUNK], F32, tag="qkp",
                                          name="qkp", bufs=5)
                            for k in range(2):
                                nc.tensor.matmul(
                                    ps[:],
                                    w_qkv[l][k][:, 128 * ot:128 * (ot + 1)],
                                    Xn[k][:, cs], start=(k == 0),
                                    stop=(k == 1))
                            if ot < 4:   # Q tiles: fold bq in the evac
                                nc.scalar.activation(
                                    qk[ot][:, cs], ps[:], AF.Identity,
                                    bias=bq_s[:, l, ot:ot + 1], scale=1.0)
                            else:
                                nc.vector.tensor_copy(out=qk[ot][:, cs],
                                                      in_=ps[:])
                    for b in range(BL):
                        bs = slice(b * L, (b + 1) * L)
                        ps = qps.tile([120, 512], F32, tag="vp", name="vp")
                        for k in range(2):
                            nc.tensor.matmul(
                                ps[:], Xn[k][:, bs],
                                w_qkv[l][k][:, 1024:1536],
                                start=(k == 0), stop=(k == 1))
                        if b % 2 == 0:
                            nc.scalar.copy(V[b][:], ps[:])
                        else:
                            nc.vector.tensor_copy(out=V[b][:], in_=ps[:])

                if l == 0 and "qkv" in dbg:
                    dq = dbg_tensor("d_q0", [512, R])
                    dk = dbg_tensor("d_k0", [512, R])
                    for ot in range(4):
                        nc.gpsimd.dma_start(
                            out=dq[128 * ot:128 * (ot + 1), :],
                            in_=qk[ot][:])
                        nc.gpsimd.dma_start(
                            out=dk[128 * ot:128 * (ot + 1), :],
                            in_=qk[4 + ot][:])
                    dv = dbg_tensor("d_v0", [BL * 120, 512])
                    for b in range(BL):
                        nc.gpsimd.dma_start(
                            out=dv[120 * b:120 * (b + 1), :], in_=V[b][:])

                if l == 0 and not dbg:
                    emit_bp_projection()

                # ---- attention (batched per head, 3-stage head pipeline)
                with contextlib.ExitStack() as ph:
                    bpl = pool(ph, f"bpl{l}", 3)
                    epl = pool(ph, f"ep{l}", 4)
                    smp = pool(ph, f"sm{l}", 2)
                    spsp = pool(ph, f"attS{l}", 2, "PSUM")
                    rtp = pool(ph, f"attRT{l}", 1, "PSUM")
                    rbtp = pool(ph, f"attRB{l}", 1, "PSUM")
                    rbp = pool(ph, f"attRb{l}", 2, "PSUM")
                    ops_ = pool(ph, f"attO{l}", 2, "PSUM")
                    st_e = {}
                    st_rbt = {}

                    def stage_a(h, l=l):
                        qt, qb = h // 2, (h % 2) * 64
                        Bph = bpl.tile([120, R], BF16, tag="bph",
                                       name="bph")
                        nc.sync.dma_start(out=Bph[:], in_=BpTd[8 * l + h])
                        # exp(s+Bp) = exp(s) * exp(Bp): precompute the bias
                        # factor once per head, multiply on DVE in bf16
                        EB = bpl.tile([120, R], BF16, tag="eb", name="eb")
                        nc.scalar.activation(EB[:], Bph[:], AF.Exp,
                                             bias=zero_col[:120, :],
                                             scale=SCALE)
                        E = epl.tile([120, R], BF16, tag="E", name="E")
                        for q in range(NCH):
                            qs = slice(q * CHUNK, (q + 1) * CHUNK)
                            sps_t = spsp.tile([120, CHUNK], F32, tag="sps",
                                              name="sps")
                            for bi in range(4):
                                b = q * 4 + bi
                                bs = slice(b * L, (b + 1) * L)
                                nc.tensor.matmul(
                                    sps_t[:, bi * L:(bi + 1) * L],
                                    qk[4 + qt][qb:qb + 64, bs],
                                    qk[qt][qb:qb + 64, bs],
                                    start=True, stop=True)
                            nc.scalar.activation(
                                E[:, qs], sps_t[:], AF.Exp,
                                bias=zero_col[:120, :], scale=SCALE)
                            nc.vector.tensor_mul(out=E[:, qs], in0=E[:, qs],
                                                 in1=EB[:, qs])
                        st_e[h] = E

                    def stage_b(h):
                        E = st_e[h]
                        rT = rtp.tile([120, 16], F32, tag="rT", name="rT")
                        for b in range(BL):
                            nc.tensor.matmul(
                                rT[:, b:b + 1], E[:, b * L:(b + 1) * L],
                                onec[:120, :], start=True, stop=True)
                        rTi = smp.tile([120, 16], BF16, tag="rTi",
                                       name="rTi")
                        with nc.allow_low_precision(
                                reason="softmax 1/Z in bf16 feeds bf16 "
                                       "matmul broadcast"):
                            nc.vector.reciprocal(out=rTi[:], in_=rT[:])
                        rbt_ps = rbtp.tile([16, 120], F32, tag="rbt",
                                           name="rbt")
                        nc.tensor.matmul(rbt_ps[:], rTi[:],
                                         eye_s[:120, :120],
                                         start=True, stop=True)
                        rbt = smp.tile([16, 120], BF16, tag="rbs",
                                       name="rbs")
                        nc.scalar.copy(rbt[:], rbt_ps[:])
                        st_rbt[h] = rbt

                    def stage_c(h):
                        qt, qb = h // 2, (h % 2) * 64
                        E, rbt = st_e.pop(h), st_rbt.pop(h)
                        for q in range(NCH):
                            qs = slice(q * CHUNK, (q + 1) * CHUNK)
                            rb_t = rbp.tile([64, CHUNK], F32, tag="rbq",
                                            name="rbq")
                            o_t = ops_.tile([64, CHUNK], F32, tag="opq",
                                            name="opq")
                            for bi in range(4):
                                b = q * 4 + bi
                                nc.tensor.matmul(
                                    rb_t[:, bi * L:(bi + 1) * L],
                                    obt_s[:, 64 * b:64 * (b + 1)], rbt[:],
                                    start=True, stop=True)
                                nc.tensor.matmul(
                                    o_t[:, bi * L:(bi + 1) * L],
                                    V[b][:, 64 * h:64 * (h + 1)],
                                    E[:, b * L:(b + 1) * L],
                                    start=True, stop=True)
                            rb_sb = smp.tile([64, CHUNK], BF16, tag="rbb",
                                             name="rbb", bufs=4)
                            if (h + q) % 2 == 0:
                                nc.scalar.copy(rb_sb[:], rb_t[:])
                            else:
                                nc.vector.tensor_copy(out=rb_sb[:],
                                                      in_=rb_t[:])
                            nc.vector.tensor_mul(
                                out=O[qt][qb:qb + 64, qs], in0=o_t[:],
                                in1=rb_sb[:])

                    for h in range(H):
                        stage_a(h)
                        if h >= 1:
                            stage_b(h - 1)
                        if h >= 2:
                            stage_c(h - 2)
                    stage_b(H - 1)
                    stage_c(H - 2)
                    stage_c(H - 1)
                qa.close()   # free qk/V
                if l == 0 and "att" in dbg:
                    do = dbg_tensor("d_att0", [512, R])
                    for ot in range(4):
                        nc.gpsimd.dma_start(
                            out=do[128 * ot:128 * (ot + 1), :], in_=O[ot][:])

                # ---- Wo + residual
                Xa = [xp.tile([128, R], F32, tag=f"X{t}", name=f"X{t}")
                      for t in range(2)]
                with contextlib.ExitStack() as ph:
                    wps = pool(ph, f"wo_ps{l}", 6, "PSUM")
                    for c in range(NCH):
                        cs = slice(c * CHUNK, (c + 1) * CHUNK)
                        for ot in range(2):
                            ps = wps.tile([128, CHUNK], F32, tag="wop",
                                          name="wop")
                            for k in range(4):
                                nc.tensor.matmul(
                                    ps[:],
                                    w_o[l][k][:, 128 * ot:128 * (ot + 1)],
                                    O[k][:, cs], start=(k == 0),
                                    stop=(k == 3))
                            nc.vector.scalar_tensor_tensor(
                                out=Xa[ot][:, cs], in0=ps[:],
                                scalar=bo_eff[:, l, ot:ot + 1],
                                in1=Xc[ot][:, cs], op0=AX.add, op1=AX.add)
                if l == 0 and "wo" in dbg:
                    d = dbg_tensor("d_wo0", [DD, R])
                    for t in range(2):
                        nc.sync.dma_start(out=d[128 * t:128 * (t + 1), :],
                                          in_=Xa[t][:])

                # ---- LN2 + gated FFN (pad grid zeroed here: fills queue
                # slack long before the conv block reads it)
                gp = pool(lp, f"grid{l}", 1)
                pad = [gp.tile([128, BL * PADC], BF16, tag=f"pad{t}",
                               name=f"pad{t}") for t in range(2)]
                nc.vector.memset(pad[0][:], 0.0)
                nc.gpsimd.memset(pad[1][:], 0.0)
                Xf = [xp.tile([128, R], F32, tag=f"X{t}", name=f"X{t}")
                      for t in range(2)]
                with contextlib.ExitStack() as ph:
                    xnp = pool(ph, f"xn2{l}", 1)
                    Xn2 = [xnp.tile([128, R], BF16, tag=f"xn2{t}",
                                    name=f"xn2{t}") for t in range(2)]

                    def w_ln2(t, c, xc, rsb, l=l, Xn2=Xn2):
                        tgt = Xn2[t][:, c * CHUNK:(c + 1) * CHUNK]
                        nc.vector.tensor_mul(out=xc[:], in0=xc[:],
                                             in1=rsb[:])
                        nc.scalar.activation(
                            tgt, xc[:], AF.Identity,
                            bias=lnb[:, 2 * l + 1, t:t + 1],
                            scale=lng[:, 2 * l + 1, t:t + 1])

                    with contextlib.ExitStack() as lnx:
                        layernorm(lnx, 2 * l + 1, Xa, NCH, CHUNK, w_ln2)

                    hp_ = pool(ph, f"hh{l}", 1)
                    Hh = [hp_.tile([128, R], BF16, tag=f"h{ot}",
                                   name=f"h{ot}") for ot in range(4)]
                    fps = pool(ph, f"f1ps{l}", 3, "PSUM")
                    fsb = pool(ph, f"f1sb{l}", 3)
                    for c in range(NCH):
                        cs = slice(c * CHUNK, (c + 1) * CHUNK)
                        for ot in range(4):
                            ps = fps.tile([128, CHUNK], F32, tag="f1a",
                                          name="f1a")
                            for k in range(2):
                                nc.tensor.matmul(
                                    ps[:],
                                    w_f1[l][k][:, 128 * ot:128 * (ot + 1)],
                                    Xn2[k][:, cs], start=(k == 0),
                                    stop=(k == 1))
                            ga = fsb.tile([128, CHUNK], BF16, tag="ga",
                                          name="ga")
                            nc.scalar.activation(
                                ga[:], ps[:], AF.Gelu,
                                bias=b1_s[:, l, ot:ot + 1], scale=1.0)
                            ps2 = fps.tile([128, CHUNK], F32, tag="f1g",
                                           name="f1g")
                            for k in range(2):
                                nc.tensor.matmul(
                                    ps2[:],
                                    w_f1[l][k][:, 512 + 128 * ot:
                                               512 + 128 * (ot + 1)],
                                    Xn2[k][:, cs], start=(k == 0),
                                    stop=(k == 1))
                            nc.vector.scalar_tensor_tensor(
                                out=Hh[ot][:, cs], in0=ps2[:],
                                scalar=b1_s[:, l, 4 + ot:5 + ot],
                                in1=ga[:], op0=AX.add, op1=AX.mult)
                    f2ps = pool(ph, f"f2ps{l}", 2, "PSUM")
                    for c in range(NCH):
                        cs = slice(c * CHUNK, (c + 1) * CHUNK)
                        for ot in range(2):
                            ps = f2ps.tile([128, CHUNK], F32, tag="f2p",
                                           name="f2p")
                            for k in range(4):
                                nc.tensor.matmul(
                                    ps[:],
                                    w_f2[l][k][:, 128 * ot:128 * (ot + 1)],
                                    Hh[k][:, cs], start=(k == 0),
                                    stop=(k == 3))
                            nc.vector.scalar_tensor_tensor(
                                out=Xf[ot][:, cs], in0=ps[:],
                                scalar=b2_s[:, l, ot:ot + 1],
                                in1=Xa[ot][:, cs], op0=AX.add, op1=AX.add)
                if l == 0 and "ffn" in dbg:
                    d = dbg_tensor("d_ffn0", [DD, R])
                    for t in range(2):
                        nc.sync.dma_start(out=d[128 * t:128 * (t + 1), :],
                                          in_=Xf[t][:])

                # ---- conv block (shared params)
                with contextlib.ExitStack() as ph:
                    Gt = [gp.tile([128, RG], F32, tag=f"g{t}", name=f"g{t}")
                          for t in range(2)]
                    for t in range(2):
                        xv = Xf[t][:].rearrange("p (b l) -> p b l", b=BL)
                        gv = Gt[t][:].rearrange("p (b c) -> p b c", b=BL)
                        for (ts_, cs_, ln_) in runs:
                            if t == 0:
                                nc.scalar.copy(gv[:, :, cs_:cs_ + ln_],
                                               xv[:, :, ts_:ts_ + ln_])
                            else:
                                nc.vector.tensor_copy(
                                    out=gv[:, :, cs_:cs_ + ln_],
                                    in_=xv[:, :, ts_:ts_ + ln_])
                        for (cs_, ln_) in uruns:
                            nc.scalar.activation(
                                gv[:, :, cs_:cs_ + ln_],
                                xv[:, :, 0:ln_], AF.Identity,
                                bias=p_s[:, t:t + 1], scale=0.0)

                    for cv in range(LC):
                        dil = DILS[cv]

                        def w_cln(t, c, xc, rsb, cv=cv):
                            b0 = c * 2
                            pv = pad[t][:].rearrange(
                                "p (b y x) -> p b y x", y=SP, x=SP)
                            nc.vector.tensor_mul(out=xc[:], in0=xc[:],
                                                 in1=rsb[:])
                            xcv = xc[:].rearrange(
                                "p (b y x) -> p b y x", y=G, x=G)
                            nc.vector.tensor_scalar(
                                out=pv[:, b0:b0 + 2, 4:16, 4:16],
                                in0=xcv[:],
                                scalar1=lng[:, 6 + cv, t:t + 1],
                                scalar2=lnb[:, 6 + cv, t:t + 1],
                                op0=AX.mult, op1=AX.add)

                        with contextlib.ExitStack() as lnx:
                            layernorm(lnx, 6 + cv, Gt, RG // GCH, GCH,
                                      w_cln)
                        with contextlib.ExitStack() as cvx:
                            cps = pool(cvx, f"cvps{l}{cv}", 6, "PSUM")
                            csb = pool(cvx, f"cvsb{l}{cv}", 4)
                            chunks = [(0, 3), (3, 3), (6, 3),
                                      (9, 3), (12, 3), (15, 1)]
                            for grp in range(2):
                                gch = chunks[3 * grp:3 * (grp + 1)]
                                pss = {}
                                for ot in range(2):
                                    for ci in range(3):
                                        pss[(ot, ci)] = cps.tile(
                                            [128, 432], F32, tag="cvp",
                                            name="cvp")
                                for ot in range(2):
                                    for tap in range(9):
                                        ky, kx = tap // 3, tap % 3
                                        y0 = 4 + (ky - 1) * dil
                                        x0 = 4 + (kx - 1) * dil
                                        for k in range(2):
                                            lhs = w_cv[cv][k][
                                                :, tap * DD + 128 * ot:
                                                tap * DD + 128 * (ot + 1)]
                                            for ci, (b0, nb) in \
                                                    enumerate(gch):
                                                pv = pad[k][:].rearrange(
                                                    "p (b y x) -> p b y x",
                                                    y=SP, x=SP)
                                                rhs = pv[:, b0:b0 + nb,
                                                         y0:y0 + G,
                                                         x0:x0 + G]
                                                nc.tensor.matmul(
                                                    pss[(ot, ci)]
                                                    [:, :nb * CELLS],
                                                    lhs, rhs,
                                                    start=(tap == 0 and
                                                           k == 0),
                                                    stop=(tap == 8 and
                                                          k == 1))
                                for ot in range(2):
                                    for ci, (b0, nb) in enumerate(gch):
                                        gs = slice(b0 * CELLS,
                                                   (b0 + nb) * CELLS)
                                        nn_ = nb * CELLS
                                        tt = csb.tile([128, 432], BF16,
                                                      tag="cvt", name="cvt")
                                        nc.scalar.activation(
                                            tt[:, :nn_],
                                            pss[(ot, ci)][:, :nn_],
                                            AF.Gelu,
                                            bias=cb_s[:, cv, ot:ot + 1],
                                            scale=1.0)
                                        nc.vector.tensor_add(
                                            out=Gt[ot][:, gs],
                                            in0=Gt[ot][:, gs],
                                            in1=tt[:, :nn_])

                    # gather tokens back
                    Xo = [xp.tile([128, R], F32, tag=f"X{t}", name=f"X{t}")
                          for t in range(2)]
                    for t in range(2):
                        xv = Xo[t][:].rearrange("p (b l) -> p b l", b=BL)
                        gv = Gt[t][:].rearrange("p (b c) -> p b c", b=BL)
                        for (ts_, cs_, ln_) in runs:
                            if t == 0:
                                nc.scalar.copy(xv[:, :, ts_:ts_ + ln_],
                                               gv[:, :, cs_:cs_ + ln_])
                            else:
                                nc.vector.tensor_copy(
                                    out=xv[:, :, ts_:ts_ + ln_],
                                    in_=gv[:, :, cs_:cs_ + ln_])
                    X.append(Xo)
                if l == 0 and "conv" in dbg:
                    d = dbg_tensor("d_conv0", [DD, R])
                    for t in range(2):
                        nc.sync.dma_start(out=d[128 * t:128 * (t + 1), :],
                                          in_=X[l + 1][t][:])

        for t in range(2):
            nc.sync.dma_start(out=outD[128 * t:128 * (t + 1), :],
                              in_=X[NL][t][:])

    _legalize_sync(nc)
    return nc, dbg_outs


# ---------------------------------------------------------------- host side
def _bf(a):
    return np.asarray(a, dtype=np.float32).astype(ml_dtypes.bfloat16)


def _pcols(vec, ncols):
    """[ncols*128] -> [128, ncols], col j = vec[j*128:(j+1)*128]."""
    return np.ascontiguousarray(
        np.asarray(vec, dtype=np.float32).reshape(ncols, 128).T)


def prepare_inputs(inputs):
    X = np.asarray(inputs["X"], dtype=np.float32)
    S = np.asarray(inputs["S_state"], dtype=np.float32)
    bias = np.asarray(inputs["bias"], dtype=np.float32)
    tok2grid = np.asarray(inputs["tok2grid"]).astype(np.int64)
    Wb = np.asarray(inputs["Wb"], dtype=np.float32)
    Wq = np.asarray(inputs["Wq"], dtype=np.float32)
    Wk = np.asarray(inputs["Wk"], dtype=np.float32)
    Wv = np.asarray(inputs["Wv"], dtype=np.float32)
    Wo_ = np.asarray(inputs["Wo"], dtype=np.float32)
    fc1_W = np.asarray(inputs["fc1_W"], dtype=np.float32)
    fc2_W = np.asarray(inputs["fc2_W"], dtype=np.float32)
    conv_W = np.asarray(inputs["conv_W"], dtype=np.float32)
    bq = np.asarray(inputs["bq"], dtype=np.float32)
    bv = np.asarray(inputs["bv"], dtype=np.float32)

    shared = {}
    # block-diagonal Wb over m-chunks of 8: [(d,mi), (h,mj)] per layer
    wblk = np.zeros([NL, 8, DB, H, 8], np.float32)
    for mi in range(8):
        wblk[:, mi, :, :, mi] = Wb
    shared["wbblk"] = _bf(np.ascontiguousarray(
        wblk.transpose(0, 2, 1, 3, 4).reshape(NL, 128, 64)
        .transpose(1, 0, 2)))
    shared["wqkv"] = _bf(np.concatenate(
        [Wq.transpose(0, 2, 1, 3).reshape(NL, DD, H * DA),
         Wk.transpose(0, 2, 1, 3).reshape(NL, DD, H * DA),
         Wv.transpose(0, 2, 1, 3).reshape(NL, DD, H * DM)], axis=2))
    shared["wo"] = _bf(Wo_)
    shared["fc1w"] = _bf(fc1_W)
    shared["fc2w"] = _bf(fc2_W)
    cw = conv_W.transpose(0, 2, 3, 4, 1).reshape(LC, DD, 9 * DD)
    shared["cww"] = _bf(cw.reshape(LC, 2, 128, 9 * DD))

    lng = np.stack([
        *[_pcols(inputs["ln1_g"][l], 2) for l in range(NL)],
        *[_pcols(inputs["ln2_g"][l], 2) for l in range(NL)],
        *[_pcols(inputs["cln_g"][c], 2) for c in range(LC)]])
    lnbv = np.stack([
        *[_pcols(inputs["ln1_b"][l], 2) for l in range(NL)],
        *[_pcols(inputs["ln2_b"][l], 2) for l in range(NL)],
        *[_pcols(inputs["cln_b"][c], 2) for c in range(LC)]])
    order = [0, 3, 1, 4, 2, 5, 6, 7, 8]
    shared["lngP"] = np.ascontiguousarray(lng[order].transpose(1, 0, 2))
    shared["lnbP"] = np.ascontiguousarray(lnbv[order].transpose(1, 0, 2))
    shared["boP"] = np.ascontiguousarray(np.stack(
        [_pcols(inputs["bo"][l], 2) for l in range(NL)], axis=1))
    shared["b1P"] = np.ascontiguousarray(np.stack(
        [_pcols(inputs["fc1_b"][l], 8) for l in range(NL)], axis=1))
    shared["b2P"] = np.ascontiguousarray(np.stack(
        [_pcols(inputs["fc2_b"][l], 2) for l in range(NL)], axis=1))
    shared["cbP"] = np.ascontiguousarray(np.stack(
        [_pcols(inputs["conv_b"][c], 2) for c in range(NL)], axis=1))
    shared["pP"] = _pcols(inputs["P"], 2)
    bqf = bq.reshape(NL, H * DA)
    shared["bqP"] = np.ascontiguousarray(np.stack(
        [_pcols(bqf[l], 4) for l in range(NL)], axis=1))
    bvf = bv.reshape(NL, H * DM)
    shared["bvP"] = _bf(np.ascontiguousarray(np.stack(
        [_pcols(bvf[l], 4) for l in range(NL)], axis=1)))
    shared["eyeD"] = _bf(np.eye(128, dtype=np.float32))
    obt = np.zeros([16, 16, 64], np.float32)
    for b in range(16):
        obt[b, b, :] = 1.0
    shared["obtD"] = _bf(obt.reshape(16, 1024))

    in_maps = []
    for c in range(NCORES):
        bs = slice(c * BL, (c + 1) * BL)
        m = dict(shared)
        rs2 = 1.0 / math.sqrt(2.0)
        m["xT"] = np.ascontiguousarray(
            X[bs].transpose(2, 0, 1).reshape(DD, R)) * rs2
        m["sT"] = np.ascontiguousarray(
            S[bs].transpose(2, 0, 1).reshape(DD, R)) * rs2
        # [mc, (d,mi), (b,l)]: biasT2[mc, d*8+mi, b*L+l] = bias[b,l,8mc+mi,d]
        bt2 = bias[bs].transpose(2, 3, 0, 1)          # [m, d, b, l]
        bt2 = bt2.reshape(MC, 8, DB, BL, L)           # [mc, mi, d, b, l]
        bt2 = bt2.transpose(0, 2, 1, 3, 4).reshape(MC, 128, BL * L)
        m["biasT2"] = _bf(np.ascontiguousarray(bt2))
        in_maps.append(m)
    return in_maps, tok2grid


_GRAPH_CACHE = {}


def kernel(**inputs):
    in_maps, tok2grid = prepare_inputs(inputs)
    key = tuple(int(v) for v in tok2grid)
    if key not in _GRAPH_CACHE:
        _GRAPH_CACHE[key] = build_graph(tok2grid)[0]
    nc = _GRAPH_CACHE[key]
    res = run_bass_kernel_spmd(nc, in_maps, core_ids=list(range(NCORES)))
    outs = []
    for c in range(NCORES):
        o = res.results[c]["out"].reshape(DD, BL, L).transpose(1, 2, 0)
        outs.append(o)
    return np.ascontiguousarray(np.concatenate(outs, axis=0),
                                dtype=np.float32)


# revision 37
# speedup vs baseline: 1.0024x; 1.0024x over previous
"""AlphaQubit-like decoder on 8 TRN2 NeuronCores (Bass/Tile).

Sharding: data-parallel over batch (16 per core), zero collectives.
Activations are feature-major [d (partitions), (b, l) (free)] so every
matmul contracts on partitions with weights stationary and every bias is
per-partition. bf16 matmul operands; f32 residual stream + PSUM.

Structure (2.21ms -> 1.46ms vs the per-(b,h) baseline):
  - bq folded into Q at QKV PSUM evacuation (per-partition bias), bk
    dropped (softmax-invariant), bv folded via bo_eff = bo + Wo^T bv;
    the (X+S)/sqrt2 prescale is applied host-side.
  - Pairwise-bias projection Bp computed with a block-diagonal lhs over
    m-chunks of 8 so the output lands m-major ([*(h,mj), (b,l)]), then
    stored to DRAM as [NH, m, (b,l)] with fat contiguous rows; attention
    loads one [120, 1920] head tile per (l,h) with 3.8KB descriptors
    (the v1 thin-row transpose loads burned ~700us of DMA-queue time).
  - Attention batched per (l,h) in a 3-stage head pipeline: S^T in
    [120,480] PSUM quarters (+Bp via identity matmul), one exp per
    quarter; row-sums computed TRANSPOSED ([120(l),16(b)] via E-as-lhs
    N=1 matmuls) so the DVE reciprocal (6.3ns/free-elem!) runs on 16
    free elems instead of 46080; orientation restored by an
    identity-rhs transpose matmul + delta-broadcast matmuls; one DVE
    multiply per quarter normalizes A@V.
  - LayerNorm rs = exp(-0.5*ln(var+eps)) on the Activation engine (ln
    and exp share one act table with attention's exp; no DVE
    reciprocal); squares on DVE in bf16; mean/rs broadcast by K=1
    matmuls; chunk-staggered emission keeps the scalar rsqrt tail off
    the PE queue's head.
  - FFN gate: (fc1g + b) * gelu(fc1a + b) via one scalar_tensor_tensor.
  - Conv LN writer uses 3D-AP activations; conv matmuls in 3-batch
    N=432 chunks; scatter/gather and pad-zeroing split across engines
    and hoisted off the conv-entry critical path.
  - Bulk DMAs rotate over gpsimd/scalar queues; sync is reserved for
    bias-tile and Bp head loads so they never queue behind stores.
"""

import contextlib
import itertools
import math

import numpy as np
import ml_dtypes

import concourse.bass as bass
import concourse.mybir as mybir
import concourse.tile as tile
from concourse.bass_utils import run_bass_kernel_spmd

F32 = mybir.dt.float32
BF16 = mybir.dt.bfloat16
AX = mybir.AluOpType
AF = mybir.ActivationFunctionType

B, L, DD = 128, 120, 256
DA, DM, H, DB = 64, 64, 8, 16
NL, LC = 3, 3
DILS = (1, 2, 4)
G = 12
NCORES = 8
BL = B // NCORES            # 16 batches per core
R = BL * L                  # 1920 seq cols per core
CELLS = G * G               # 144
RG = BL * CELLS             # 2304 grid cols per core
NH = NL * H                 # 24 (layer,head) rows
SP = 20                     # padded grid side (pad=4 fits dilation<=4)
PADC = SP * SP
CHUNK = 480                 # seq chunk (4 batches)
NCH = R // CHUNK            # 4 seq chunks
GCH = 288                   # grid chunk (2 batches)
MC = 15                     # m-chunks of 8 in the Bp projection
EPS = 1e-5
SCALE = 1.0 / math.sqrt(DA)


def _legalize_sync(nc):
    """This walrus build caps per-instruction semaphore waits (drain: 0,
    matmul/ldweights: 1, others: 2); move excess waits onto NoOps."""
    caps = {"InstDrain": 0}
    for f in nc.m.functions:
        for bb in f.blocks:
            new_insts = []
            for inst in bb.instructions:
                si = getattr(inst, "sync_info", None)
                cap = caps.get(type(inst).__name__, 1)
                if si is not None and si.on_wait and len(si.on_wait) > cap:
                    waits = list(si.on_wait)
                    keep = waits[len(waits) - cap:] if cap else []
                    for i, w in enumerate(waits[: len(waits) - cap]):
                        new_insts.append(mybir.InstNoOp(
                            name=inst.name + f"-ws{i}", engine=inst.engine,
                            ins=[], outs=[],
                            sync_info=mybir.SyncInfo(on_wait=[w],
                                                     on_update=[])))
                    si.on_wait = keep
                new_insts.append(inst)
            bb.instructions = new_insts


def build_graph(tok2grid, debug_taps=()):
    dbg = set(debug_taps)
    nc = bass.Bass()

    xT = nc.dram_tensor("xT", [DD, R], F32, kind="ExternalInput")
    sT = nc.dram_tensor("sT", [DD, R], F32, kind="ExternalInput")
    biasT2 = nc.dram_tensor("biasT2", [MC, 128, R], BF16,
                            kind="ExternalInput")
    wbblk = nc.dram_tensor("wbblk", [128, NL, 64], BF16,
                           kind="ExternalInput")
    wqkv = nc.dram_tensor("wqkv", [NL, DD, 1536], BF16, kind="ExternalInput")
    wo = nc.dram_tensor("wo", [NL, H * DM, DD], BF16, kind="ExternalInput")
    fc1w = nc.dram_tensor("fc1w", [NL, DD, 1024], BF16, kind="ExternalInput")
    fc2w = nc.dram_tensor("fc2w", [NL, 512, DD], BF16, kind="ExternalInput")
    cww = nc.dram_tensor("cww", [LC, 2, 128, 9 * DD], BF16,
                         kind="ExternalInput")
    lngP = nc.dram_tensor("lngP", [128, 9, 2], F32, kind="ExternalInput")
    lnbP = nc.dram_tensor("lnbP", [128, 9, 2], F32, kind="ExternalInput")
    boP = nc.dram_tensor("boP", [128, NL, 2], F32, kind="ExternalInput")
    b1P = nc.dram_tensor("b1P", [128, NL, 8], F32, kind="ExternalInput")
    b2P = nc.dram_tensor("b2P", [128, NL, 2], F32, kind="ExternalInput")
    cbP = nc.dram_tensor("cbP", [128, LC, 2], F32, kind="ExternalInput")
    pP = nc.dram_tensor("pP", [128, 2], F32, kind="ExternalInput")
    bqP = nc.dram_tensor("bqP", [128, NL, 4], F32, kind="ExternalInput")
    bvP = nc.dram_tensor("bvP", [128, NL, 4], BF16, kind="ExternalInput")
    eyeD = nc.dram_tensor("eyeD", [128, 128], BF16, kind="ExternalInput")
    obtD = nc.dram_tensor("obtD", [16, 1024], BF16, kind="ExternalInput")
    outD = nc.dram_tensor("out", [DD, R], F32, kind="ExternalOutput")

    dbg_outs = {}

    def dbg_tensor(name, shape):
        dbg_outs[name] = nc.dram_tensor(name, shape, F32,
                                        kind="ExternalOutput")
        return dbg_outs[name]

    # token<->grid runs (host-known permutation baked into APs)
    t2g = [int(v) for v in tok2grid]
    assert len(set(t2g)) == L and all(0 <= v < CELLS for v in t2g)
    runs = []
    s = 0
    for i in range(1, L + 1):
        if i == L or t2g[i] != t2g[i - 1] + 1:
            runs.append((s, t2g[s], i - s))
            s = i
    unmapped = sorted(set(range(CELLS)) - set(t2g))
    uruns = []
    if unmapped:
        s = 0
        for i in range(1, len(unmapped) + 1):
            if i == len(unmapped) or unmapped[i] != unmapped[i - 1] + 1:
                uruns.append((unmapped[s], i - s))
                s = i

    uid = [0]

    def pool(lctx, name, bufs, space="SBUF"):
        uid[0] += 1
        return lctx.enter_context(
            tc.tile_pool(name=f"{name}_{uid[0]}", bufs=bufs, space=space))

    # rotate bulk DMAs across engine queues
    qrr = itertools.cycle(["gpsimd", "scalar"])

    def dma(out, in_):
        getattr(nc, next(qrr)).dma_start(out=out, in_=in_)

    with tile.TileContext(nc) as tc, contextlib.ExitStack() as ctx:
        wp = ctx.enter_context(tc.tile_pool(name="weights", bufs=1))
        xp = ctx.enter_context(tc.tile_pool(name="xres", bufs=2))
        dram = ctx.enter_context(tc.tile_pool(name="dram", bufs=1,
                                              space="DRAM"))

        # ---------------- weights/constants to SBUF
        w_qkv = [[wp.tile([128, 1536], BF16, tag=f"wqkv{l}{k}",
                          name=f"wqkv{l}{k}") for k in range(2)]
                 for l in range(NL)]
        w_o = [[wp.tile([128, DD], BF16, tag=f"wo{l}{k}", name=f"wo{l}{k}")
                for k in range(4)] for l in range(NL)]
        w_f1 = [[wp.tile([128, 1024], BF16, tag=f"f1{l}{k}", name=f"f1{l}{k}")
                 for k in range(2)] for l in range(NL)]
        w_f2 = [[wp.tile([128, DD], BF16, tag=f"f2{l}{k}", name=f"f2{l}{k}")
                 for k in range(4)] for l in range(NL)]
        w_cv = [[wp.tile([128, 9 * DD], BF16, tag=f"cv{c}{k}",
                         name=f"cv{c}{k}") for k in range(2)]
                for c in range(LC)]
        for k in range(2):
            dma(w_qkv[0][k][:], wqkv[0, 128 * k:128 * (k + 1), :])
        for l in range(NL):
            for k in range(4):
                dma(w_o[l][k][:], wo[l, 128 * k:128 * (k + 1), :])

        def load_late_weights():
            for l in range(NL):
                for k in range(2):
                    if l > 0:
                        dma(w_qkv[l][k][:],
                            wqkv[l, 128 * k:128 * (k + 1), :])
                    dma(w_f1[l][k][:], fc1w[l, 128 * k:128 * (k + 1), :])
                for k in range(4):
                    dma(w_f2[l][k][:], fc2w[l, 128 * k:128 * (k + 1), :])
            for c in range(LC):
                for k in range(2):
                    dma(w_cv[c][k][:], cww[c, k, :, :])

        lng = wp.tile([128, 9, 2], F32)
        dma(lng[:], lngP[:])
        lnb = wp.tile([128, 9, 2], F32)
        dma(lnb[:], lnbP[:])
        bo_s = wp.tile([128, NL, 2], F32)
        dma(bo_s[:], boP[:])
        b1_s = wp.tile([128, NL, 8], F32)
        dma(b1_s[:], b1P[:])
        b2_s = wp.tile([128, NL, 2], F32)
        dma(b2_s[:], b2P[:])
        cb_s = wp.tile([128, LC, 2], F32)
        dma(cb_s[:], cbP[:])
        p_s = wp.tile([128, 2], F32)
        dma(p_s[:], pP[:])
        bq_s = wp.tile([128, NL, 4], F32)
        dma(bq_s[:], bqP[:])
        bv_s = wp.tile([128, NL, 4], BF16)
        dma(bv_s[:], bvP[:])
        eye_s = wp.tile([128, 128], BF16)
        dma(eye_s[:], eyeD[:])
        obt_s = wp.tile([16, 1024], BF16)
        dma(obt_s[:], obtD[:])
        wbb_s = wp.tile([128, NL, 64], BF16)
        dma(wbb_s[:], wbblk[:])

        ones_rowb = wp.tile([1, 128], BF16)
        nc.vector.memset(ones_rowb[:], 1.0)
        invd_col = wp.tile([128, 1], BF16)
        nc.vector.memset(invd_col[:], 1.0 / DD)
        onec = wp.tile([128, 1], BF16)
        nc.vector.memset(onec[:], 1.0)
        eps_col = wp.tile([128, 1], F32)
        nc.vector.memset(eps_col[:], EPS)
        zero_col = wp.tile([128, 1], F32)
        nc.vector.memset(zero_col[:], 0.0)

        # fold bv into bo: bo_eff = bo + Wo^T bv
        bo_eff = wp.tile([128, NL, 2], F32)
        with tc.tile_pool(name="prep_ps", bufs=2, space="PSUM") as pps:
            for l in range(NL):
                d_ps = pps.tile([128, 2], F32, tag="dps", name="dps")
                for ot in range(2):
                    for k in range(4):
                        nc.tensor.matmul(
                            d_ps[:, ot:ot + 1],
                            w_o[l][k][:, 128 * ot:128 * (ot + 1)],
                            bv_s[:, l, k:k + 1],
                            start=(k == 0), stop=(k == 3))
                nc.vector.tensor_add(out=bo_eff[:, l], in0=d_ps[:],
                                     in1=bo_s[:, l])

        # ---------------- residual init: x = X + S (host pre-scaled 1/sqrt2)
        X = [[xp.tile([128, R], F32, tag=f"X{t}", name=f"X{t}")
              for t in range(2)]]
        with contextlib.ExitStack() as lctx:
            ipool = pool(lctx, "init", 2)
            for t in range(2):
                xi = ipool.tile([128, R], F32, tag="xi", name="xi")
                si = ipool.tile([128, R], F32, tag="si", name="si")
                nc.sync.dma_start(out=xi[:], in_=xT[128 * t:128 * (t + 1), :])
                nc.sync.dma_start(out=si[:], in_=sT[128 * t:128 * (t + 1), :])
                nc.vector.tensor_add(out=X[0][t][:], in0=xi[:], in1=si[:])
        if "x0" in dbg:
            d = dbg_tensor("d_x0", [DD, R])
            for t in range(2):
                nc.sync.dma_start(out=d[128 * t:128 * (t + 1), :],
                                  in_=X[0][t][:])

        # ---------------- layernorm (feature-major, chunk-staggered)
        def layernorm(lctx, li, src, n_chunks, ccols, writer):
            """src: 2 f32 tiles [128, n_chunks*ccols]. writer(t, c, xc, rsb)
            must emit (xc * g) * rs + b into its target; xc f32 SBUF
            [128, ccols], rsb f32 PSUM [128, ccols] broadcast of rs.
            Stats matmuls of chunk c+1 are emitted before the broadcast of
            chunk c so the PE queue never stalls on the scalar rsqrt tail."""
            sps = pool(lctx, "ln_st", 2, "PSUM")
            bps = pool(lctx, "ln_bc", 2, "PSUM")
            sp = pool(lctx, "ln_sb", 2)

            def stats(c):
                cs = slice(c * ccols, (c + 1) * ccols)
                xb = [sp.tile([128, 2, ccols], BF16, tag=f"xb{t}",
                              name=f"xb{t}") for t in range(2)]
                for t in range(2):
                    nc.gpsimd.dma_start(out=xb[t][:, 0], in_=src[t][:, cs])
                for t in range(2):   # squares on DVE (bf16 fast mode)
                    nc.vector.tensor_mul(out=xb[t][:, 1], in0=xb[t][:, 0],
                                         in1=xb[t][:, 0])
                st = sps.tile([1, 2, 512], F32, tag="st", name="st")
                for t in range(2):
                    nc.tensor.matmul(st[:, 0, :ccols], invd_col[:],
                                     xb[t][:, 0], start=(t == 0),
                                     stop=(t == 1))
                for t in range(2):
                    nc.tensor.matmul(st[:, 1, :ccols], invd_col[:],
                                     xb[t][:, 1], start=(t == 0),
                                     stop=(t == 1))
                return st

            def tail(c, st):
                cs = slice(c * ccols, (c + 1) * ccols)
                mrs = sp.tile([1, 2, ccols], BF16, tag="mrs", name="mrs")
                nc.scalar.copy(mrs[:, 0], st[:, 0, :ccols])
                sqm = sp.tile([1, ccols], F32, tag="sqm", name="sqm")
                nc.scalar.square(sqm[:], mrs[:, 0])
                lnv = sp.tile([1, ccols], F32, tag="lnv", name="lnv")
                nc.vector.tensor_sub(out=lnv[:], in0=st[:, 1, :ccols],
                                     in1=sqm[:])
                # rs = (var+eps)^-1/2 = exp(-0.5*ln(var+eps))
                nc.scalar.activation(lnv[:], lnv[:], AF.Ln,
                                     bias=eps_col[0:1, :], scale=1.0)
                nc.scalar.activation(mrs[:, 1], lnv[:], AF.Exp,
                                     bias=zero_col[0:1, :], scale=-0.5)
                mb = bps.tile([128, ccols], F32, tag="mb", name="mb")
                nc.tensor.matmul(mb[:], ones_rowb[:], mrs[:, 0],
                                 start=True, stop=True)
                rsb = bps.tile([128, ccols], F32, tag="rsb", name="rsb")
                nc.tensor.matmul(rsb[:], ones_rowb[:], mrs[:, 1],
                                 start=True, stop=True)
                for t in range(2):
                    xc = sp.tile([128, ccols], F32, tag=f"xc{t}",
                                 name=f"xc{t}")
                    nc.vector.tensor_sub(out=xc[:], in0=src[t][:, cs],
                                         in1=mb[:])
                    writer(t, c, xc, rsb)

            prev = None
            for c in range(n_chunks):
                st = stats(c)
                if prev is not None:
                    tail(*prev)
                prev = (c, st)
            tail(*prev)

        # ---------------- Bp projection: m-major blocks -> DRAM
        BpTd = dram.tile([NH, 120, R], BF16)

        def emit_bp_projection():
            with contextlib.ExitStack() as lctx:
                btp = pool(lctx, "bp_in", 3)
                bpp = pool(lctx, "bp_ps", 8, "PSUM")
                bst = pool(lctx, "bp_st", 3)
                for mc in range(MC):
                    bt = btp.tile([128, R], BF16, tag="bt", name="bt")
                    nc.sync.dma_start(out=bt[:], in_=biasT2[mc])
                    for l in range(NL):
                        stage = bst.tile([64, R], BF16, tag="bstg",
                                         name="bstg")
                        for q in range(NCH):
                            qs = slice(q * CHUNK, (q + 1) * CHUNK)
                            ps = bpp.tile([64, CHUNK], F32, tag="bpp",
                                          name="bpp")
                            nc.tensor.matmul(ps[:], wbb_s[:, l, :],
                                             bt[:, qs],
                                             start=True, stop=True)
                            if (mc + l + q) % 2 == 0:
                                nc.scalar.copy(stage[:, qs], ps[:])
                            else:
                                nc.vector.tensor_copy(out=stage[:, qs],
                                                      in_=ps[:])
                        for h in range(H):
                            dma(BpTd[8 * l + h, 8 * mc:8 * (mc + 1), :],
                                stage[8 * h:8 * (h + 1), :])

        load_late_weights()

        if dbg:   # debug taps need the projection data eagerly
            emit_bp_projection()
        if "bp" in dbg:
            d_bp = nc.dram_tensor("d_bp", [NH, 120, R], BF16,
                                  kind="ExternalOutput")
            dbg_outs["d_bp"] = d_bp
            nc.sync.dma_start(out=d_bp[:], in_=BpTd[:])

        # ---------------- layers
        for l in range(NL):
            Xc = X[l]
            with contextlib.ExitStack() as lp:
                op_ = pool(lp, f"op{l}", 1)    # O^T: lives till Wo
                O = [op_.tile([128, R], BF16, tag=f"o{ot}", name=f"o{ot}")
                     for ot in range(4)]
                qa = contextlib.ExitStack()
                lp.enter_context(qa)
                qv = pool(qa, f"qv{l}", 1)   # qk/V live till attn end
                qk = [qv.tile([128, R], BF16, tag=f"qk{ot}",
                              name=f"qk{ot}") for ot in range(8)]
                V = [qv.tile([120, 512], BF16, tag=f"v{b}",
                             name=f"v{b}") for b in range(BL)]
                # ---- LN1 + QKV (Xn scoped)
                with contextlib.ExitStack() as ph:
                    xnp = pool(ph, f"xn{l}", 1)
                    Xn = [xnp.tile([128, R], BF16, tag=f"xn{t}",
                                   name=f"xn{t}") for t in range(2)]

                    def w_ln1(t, c, xc, rsb, l=l, Xn=Xn):
                        tgt = Xn[t][:, c * CHUNK:(c + 1) * CHUNK]
                        nc.vector.tensor_mul(out=xc[:], in0=xc[:],
                                             in1=rsb[:])
                        nc.vector.tensor_scalar(
                            out=tgt, in0=xc[:],
                            scalar1=lng[:, 2 * l, t:t + 1],
                            scalar2=lnb[:, 2 * l, t:t + 1],
                            op0=AX.mult, op1=AX.add)

                    with contextlib.ExitStack() as lnx:
                        layernorm(lnx, 2 * l, Xc, NCH, CHUNK, w_ln1)
                    if l == 0 and "ln1" in dbg:
                        d = dbg_tensor("d_ln1", [DD, R])
                        for t in range(2):
                            nc.gpsimd.dma_start(
                                out=d[128 * t:128 * (t + 1), :],
                                in_=Xn[t][:])

                    qps = pool(ph, f"qkv_ps{l}", 2, "PSUM")
                    for c in range(NCH):
                        cs = slice(c * CHUNK, (c + 1) * CHUNK)
                        for ot in range(8):
                            ps = qps.tile([128, CHUNK], F32, tag="qkp",
                                          name="qkp", bufs=5)
                            for k in range(2):
                                nc.tensor.matmul(
                                    ps[:],
                                    w_qkv[l][k][:, 128 * ot:128 * (ot + 1)],
                                    Xn[k][:, cs], start=(k == 0),
                                    stop=(k == 1))
                            if ot < 4:   # Q tiles: fold bq in the evac
                                nc.scalar.activation(
                                    qk[ot][:, cs], ps[:], AF.Identity,
                                    bias=bq_s[:, l, ot:ot + 1], scale=1.0)
                            else:
                                nc.vector.tensor_copy(out=qk[ot][:, cs],
                                                      in_=ps[:])
                    for b in range(BL):
                        bs = slice(b * L, (b + 1) * L)
                        ps = qps.tile([120, 512], F32, tag="vp", name="vp")
                        for k in range(2):
                            nc.tensor.matmul(
                                ps[:], Xn[k][:, bs],
                                w_qkv[l][k][:, 1024:1536],
                                start=(k == 0), stop=(k == 1))
                        if b % 2 == 0:
                            nc.scalar.copy(V[b][:], ps[:])
                        else:
                            nc.vector.tensor_copy(out=V[b][:], in_=ps[:])

                if l == 0 and "qkv" in dbg:
                    dq = dbg_tensor("d_q0", [512, R])
                    dk = dbg_tensor("d_k0", [512, R])
                    for ot in range(4):
                        nc.gpsimd.dma_start(
                            out=dq[128 * ot:128 * (ot + 1), :],
                            in_=qk[ot][:])
                        nc.gpsimd.dma_start(
                            out=dk[128 * ot:128 * (ot + 1), :],
                            in_=qk[4 + ot][:])
                    dv = dbg_tensor("d_v0", [BL * 120, 512])
                    for b in range(BL):
                        nc.gpsimd.dma_start(
                            out=dv[120 * b:120 * (b + 1), :], in_=V[b][:])

                if l == 0 and not dbg:
                    emit_bp_projection()

                # ---- attention (batched per head, 3-stage head pipeline)
                with contextlib.ExitStack() as ph:
                    bpl = pool(ph, f"bpl{l}", 3)
                    epl = pool(ph, f"ep{l}", 3)
                    smp = pool(ph, f"sm{l}", 2)
                    spsp = pool(ph, f"attS{l}", 2, "PSUM")
                    rtp = pool(ph, f"attRT{l}", 1, "PSUM")
                    rbtp = pool(ph, f"attRB{l}", 1, "PSUM")
                    rbp = pool(ph, f"attRb{l}", 2, "PSUM")
                    ops_ = pool(ph, f"attO{l}", 2, "PSUM")
                    st_e = {}
                    st_rbt = {}

                    def stage_a(h, l=l):
                        qt, qb = h // 2, (h % 2) * 64
                        Bph = bpl.tile([120, R], BF16, tag="bph",
                                       name="bph")
                        nc.sync.dma_start(out=Bph[:], in_=BpTd[8 * l + h])
                        # exp(s+Bp) = exp(s) * exp(Bp): precompute the bias
                        # factor once per head, multiply on DVE in bf16
                        EB = bpl.tile([120, R], BF16, tag="eb", name="eb")
                        nc.scalar.activation(EB[:], Bph[:], AF.Exp,
                                             bias=zero_col[:120, :],
                                             scale=SCALE)
                        E = epl.tile([120, R], BF16, tag="E", name="E")
                        for q in range(NCH):
                            qs = slice(q * CHUNK, (q + 1) * CHUNK)
                            sps_t = spsp.tile([120, CHUNK], F32, tag="sps",
                                              name="sps")
                            for bi in range(4):
                                b = q * 4 + bi
                                bs = slice(b * L, (b + 1) * L)
                                nc.tensor.matmul(
                                    sps_t[:, bi * L:(bi + 1) * L],
                                    qk[4 + qt][qb:qb + 64, bs],
                                    qk[qt][qb:qb + 64, bs],
                                    start=True, stop=True)
                            nc.scalar.activation(
                                E[:, qs], sps_t[:], AF.Exp,
                                bias=zero_col[:120, :], scale=SCALE)
                            nc.vector.tensor_mul(out=E[:, qs], in0=E[:, qs],
                                                 in1=EB[:, qs])
                        st_e[h] = E

                    def stage_b(h):
                        E = st_e[h]
                        rT = rtp.tile([120, 16], F32, tag="rT", name="rT")
                        for b in range(BL):
                            nc.tensor.matmul(
                                rT[:, b:b + 1], E[:, b * L:(b + 1) * L],
                                onec[:120, :], start=True, stop=True)
                        rTi = smp.tile([120, 16], BF16, tag="rTi",
                                       name="rTi")
                        with nc.allow_low_precision(
                                reason="softmax 1/Z in bf16 feeds bf16 "
                                       "matmul broadcast"):
                            nc.vector.reciprocal(out=rTi[:], in_=rT[:])
                        rbt_ps = rbtp.tile([16, 120], F32, tag="rbt",
                                           name="rbt")
                        nc.tensor.matmul(rbt_ps[:], rTi[:],
                                         eye_s[:120, :120],
                                         start=True, stop=True)
                        rbt = smp.tile([16, 120], BF16, tag="rbs",
                                       name="rbs")
                        nc.scalar.copy(rbt[:], rbt_ps[:])
                        st_rbt[h] = rbt

                    def stage_c(h):
                        qt, qb = h // 2, (h % 2) * 64
                        E, rbt = st_e.pop(h), st_rbt.pop(h)
                        for q in range(NCH):
                            qs = slice(q * CHUNK, (q + 1) * CHUNK)
                            rb_t = rbp.tile([64, CHUNK], F32, tag="rbq",
                                            name="rbq")
                            o_t = ops_.tile([64, CHUNK], F32, tag="opq",
                                            name="opq")
                            for bi in range(4):
                                b = q * 4 + bi
                                nc.tensor.matmul(
                                    rb_t[:, bi * L:(bi + 1) * L],
                                    obt_s[:, 64 * b:64 * (b + 1)], rbt[:],
                                    start=True, stop=True)
                                nc.tensor.matmul(
                                    o_t[:, bi * L:(bi + 1) * L],
                                    V[b][:, 64 * h:64 * (h + 1)],
                                    E[:, b * L:(b + 1) * L],
                                    start=True, stop=True)
                            rb_sb = smp.tile([64, CHUNK], BF16, tag="rbb",
                                             name="rbb")
                            if (h + q) % 2 == 0:
                                nc.scalar.copy(rb_sb[:], rb_t[:])
                            else:
                                nc.vector.tensor_copy(out=rb_sb[:],
                                                      in_=rb_t[:])
                            nc.vector.tensor_mul(
                                out=O[qt][qb:qb + 64, qs], in0=o_t[:],
                                in1=rb_sb[:])

                    for h in range(H):
                        stage_a(h)
                        if h >= 1:
                            stage_b(h - 1)
                        if h >= 2:
                            stage_c(h - 2)
                    stage_b(H - 1)
                    stage_c(H - 2)
                    stage_c(H - 1)
                qa.close()   # free qk/V
                if l == 0 and "att" in dbg:
                    do = dbg_tensor("d_att0", [512, R])
                    for ot in range(4):
                        nc.gpsimd.dma_start(
                            out=do[128 * ot:128 * (ot + 1), :], in_=O[ot][:])

                # ---- Wo + residual
                Xa = [xp.tile([128, R], F32, tag=f"X{t}", name=f"X{t}")
                      for t in range(2)]
                with contextlib.ExitStack() as ph:
                    wps = pool(ph, f"wo_ps{l}", 6, "PSUM")
                    for c in range(NCH):
                        cs = slice(c * CHUNK, (c + 1) * CHUNK)
                        for ot in range(2):
                            ps = wps.tile([128, CHUNK], F32, tag="wop",
                                          name="wop")
                            for k in range(4):
                                nc.tensor.matmul(
                                    ps[:],
                                    w_o[l][k][:, 128 * ot:128 * (ot + 1)],
                                    O[k][:, cs], start=(k == 0),
                                    stop=(k == 3))
                            nc.vector.scalar_tensor_tensor(
                                out=Xa[ot][:, cs], in0=ps[:],
                                scalar=bo_eff[:, l, ot:ot + 1],
                                in1=Xc[ot][:, cs], op0=AX.add, op1=AX.add)
                if l == 0 and "wo" in dbg:
                    d = dbg_tensor("d_wo0", [DD, R])
                    for t in range(2):
                        nc.sync.dma_start(out=d[128 * t:128 * (t + 1), :],
                                          in_=Xa[t][:])

                # ---- LN2 + gated FFN (pad grid zeroed here: fills queue
                # slack long before the conv block reads it)
                gp = pool(lp, f"grid{l}", 1)
                pad = [gp.tile([128, BL * PADC], BF16, tag=f"pad{t}",
                               name=f"pad{t}") for t in range(2)]
                nc.vector.memset(pad[0][:], 0.0)
                nc.gpsimd.memset(pad[1][:], 0.0)
                Xf = [xp.tile([128, R], F32, tag=f"X{t}", name=f"X{t}")
                      for t in range(2)]
                with contextlib.ExitStack() as ph:
                    xnp = pool(ph, f"xn2{l}", 1)
                    Xn2 = [xnp.tile([128, R], BF16, tag=f"xn2{t}",
                                    name=f"xn2{t}") for t in range(2)]

                    def w_ln2(t, c, xc, rsb, l=l, Xn2=Xn2):
                        tgt = Xn2[t][:, c * CHUNK:(c + 1) * CHUNK]
                        nc.vector.tensor_mul(out=xc[:], in0=xc[:],
                                             in1=rsb[:])
                        nc.scalar.activation(
                            tgt, xc[:], AF.Identity,
                            bias=lnb[:, 2 * l + 1, t:t + 1],
                            scale=lng[:, 2 * l + 1, t:t + 1])

                    with contextlib.ExitStack() as lnx:
                        layernorm(lnx, 2 * l + 1, Xa, NCH, CHUNK, w_ln2)

                    hp_ = pool(ph, f"hh{l}", 1)
                    Hh = [hp_.tile([128, R], BF16, tag=f"h{ot}",
                                   name=f"h{ot}") for ot in range(4)]
                    fps = pool(ph, f"f1ps{l}", 3, "PSUM")
                    fsb = pool(ph, f"f1sb{l}", 3)
                    for c in range(NCH):
                        cs = slice(c * CHUNK, (c + 1) * CHUNK)
                        for ot in range(4):
                            ps = fps.tile([128, CHUNK], F32, tag="f1a",
                                          name="f1a")
                            for k in range(2):
                                nc.tensor.matmul(
                                    ps[:],
                                    w_f1[l][k][:, 128 * ot:128 * (ot + 1)],
                                    Xn2[k][:, cs], start=(k == 0),
                                    stop=(k == 1))
                            ga = fsb.tile([128, CHUNK], BF16, tag="ga",
                                          name="ga")
                            nc.scalar.activation(
                                ga[:], ps[:], AF.Gelu,
                                bias=b1_s[:, l, ot:ot + 1], scale=1.0)
                            ps2 = fps.tile([128, CHUNK], F32, tag="f1g",
                                           name="f1g")
                            for k in range(2):
                                nc.tensor.matmul(
                                    ps2[:],
                                    w_f1[l][k][:, 512 + 128 * ot:
                                               512 + 128 * (ot + 1)],
                                    Xn2[k][:, cs], start=(k == 0),
                                    stop=(k == 1))
                            nc.vector.scalar_tensor_tensor(
                                out=Hh[ot][:, cs], in0=ps2[:],
                                scalar=b1_s[:, l, 4 + ot:5 + ot],
                                in1=ga[:], op0=AX.add, op1=AX.mult)
                    f2ps = pool(ph, f"f2ps{l}", 2, "PSUM")
                    for c in range(NCH):
                        cs = slice(c * CHUNK, (c + 1) * CHUNK)
                        for ot in range(2):
                            ps = f2ps.tile([128, CHUNK], F32, tag="f2p",
                                           name="f2p")
                            for k in range(4):
                                nc.tensor.matmul(
                                    ps[:],
                                    w_f2[l][k][:, 128 * ot:128 * (ot + 1)],
                                    Hh[k][:, cs], start=(k == 0),
                                    stop=(k == 3))
                            nc.vector.scalar_tensor_tensor(
                                out=Xf[ot][:, cs], in0=ps[:],
                                scalar=b2_s[:, l, ot:ot + 1],
                                in1=Xa[ot][:, cs], op0=AX.add, op1=AX.add)
                if l == 0 and "ffn" in dbg:
                    d = dbg_tensor("d_ffn0", [DD, R])
                    for t in range(2):
                        nc.sync.dma_start(out=d[128 * t:128 * (t + 1), :],
                                          in_=Xf[t][:])

                # ---- conv block (shared params)
                with contextlib.ExitStack() as ph:
                    Gt = [gp.tile([128, RG], F32, tag=f"g{t}", name=f"g{t}")
                          for t in range(2)]
                    for t in range(2):
                        xv = Xf[t][:].rearrange("p (b l) -> p b l", b=BL)
                        gv = Gt[t][:].rearrange("p (b c) -> p b c", b=BL)
                        for (ts_, cs_, ln_) in runs:
                            if t == 0:
                                nc.scalar.copy(gv[:, :, cs_:cs_ + ln_],
                                               xv[:, :, ts_:ts_ + ln_])
                            else:
                                nc.vector.tensor_copy(
                                    out=gv[:, :, cs_:cs_ + ln_],
                                    in_=xv[:, :, ts_:ts_ + ln_])
                        for (cs_, ln_) in uruns:
                            nc.scalar.activation(
                                gv[:, :, cs_:cs_ + ln_],
                                xv[:, :, 0:ln_], AF.Identity,
                                bias=p_s[:, t:t + 1], scale=0.0)

                    for cv in range(LC):
                        dil = DILS[cv]

                        def w_cln(t, c, xc, rsb, cv=cv):
                            b0 = c * 2
                            pv = pad[t][:].rearrange(
                                "p (b y x) -> p b y x", y=SP, x=SP)
                            nc.vector.tensor_mul(out=xc[:], in0=xc[:],
                                                 in1=rsb[:])
                            xcv = xc[:].rearrange(
                                "p (b y x) -> p b y x", y=G, x=G)
                            nc.vector.tensor_scalar(
                                out=pv[:, b0:b0 + 2, 4:16, 4:16],
                                in0=xcv[:],
                                scalar1=lng[:, 6 + cv, t:t + 1],
                                scalar2=lnb[:, 6 + cv, t:t + 1],
                                op0=AX.mult, op1=AX.add)

                        with contextlib.ExitStack() as lnx:
                            layernorm(lnx, 6 + cv, Gt, RG // GCH, GCH,
                                      w_cln)
                        with contextlib.ExitStack() as cvx:
                            cps = pool(cvx, f"cvps{l}{cv}", 6, "PSUM")
                            csb = pool(cvx, f"cvsb{l}{cv}", 4)
                            chunks = [(0, 3), (3, 3), (6, 3),
                                      (9, 3), (12, 3), (15, 1)]
                            for grp in range(2):
                                gch = chunks[3 * grp:3 * (grp + 1)]
                                pss = {}
                                for ot in range(2):
                                    for ci in range(3):
                                        pss[(ot, ci)] = cps.tile(
                                            [128, 432], F32, tag="cvp",
                                            name="cvp")
                                for ot in range(2):
                                    for tap in range(9):
                                        ky, kx = tap // 3, tap % 3
                                        y0 = 4 + (ky - 1) * dil
                                        x0 = 4 + (kx - 1) * dil
                                        for k in range(2):
                                            lhs = w_cv[cv][k][
                                                :, tap * DD + 128 * ot:
                                                tap * DD + 128 * (ot + 1)]
                                            for ci, (b0, nb) in \
                                                    enumerate(gch):
                                                pv = pad[k][:].rearrange(
                                                    "p (b y x) -> p b y x",
                                                    y=SP, x=SP)
                                                rhs = pv[:, b0:b0 + nb,
                                                         y0:y0 + G,
                                                         x0:x0 + G]
                                                nc.tensor.matmul(
                                                    pss[(ot, ci)]
                                                    [:, :nb * CELLS],
                                                    lhs, rhs,
                                                    start=(tap == 0 and
                                                           k == 0),
                                                    stop=(tap == 8 and
                                                          k == 1))
                                for ot in range(2):
                                    for ci, (b0, nb) in enumerate(gch):
                                        gs = slice(b0 * CELLS,
                                                   (b0 + nb) * CELLS)
                                        nn_ = nb * CELLS
                                        tt = csb.tile([128, 432], BF16,
                                                      tag="cvt", name="cvt")
                                        nc.scalar.activation(
                                            tt[:, :nn_],
                                            pss[(ot, ci)][:, :nn_],
                                            AF.Gelu,
                                            bias=cb_s[:, cv, ot:ot + 1],
                                            scale=1.0)
                                        nc.vector.tensor_add(
                                            out=Gt[ot][:, gs],
                                            in0=Gt[ot][:, gs],
                                            in1=tt[:, :nn_])

                    # gather tokens back
                    Xo = [xp.tile([128, R], F32, tag=f"X{t}", name=f"X{t}")
                          for t in range(2)]
                    for t in range(2):
                        xv = Xo[t][:].rearrange("p (b l) -> p b l", b=BL)
                        gv = Gt[t][:].rearrange("p (b c) -> p b c", b=BL)
                        for (ts_, cs_, ln_) in runs:
                            if t == 0:
                                nc.scalar.copy(xv[:, :, ts_:ts_ + ln_],
                                               gv[:, :, cs_:cs_ + ln_])
                            else:
                                nc.vector.tensor_copy(
                                    out=xv[:, :, ts_:ts_ + ln_],
                                    in_=gv[:, :, cs_:cs_ + ln_])
                    X.append(Xo)
                if l == 0 and "conv" in dbg:
                    d = dbg_tensor("d_conv0", [DD, R])
                    for t in range(2):
                        nc.sync.dma_start(out=d[128 * t:128 * (t + 1), :],
                                          in_=X[l + 1][t][:])

        for t in range(2):
            nc.sync.dma_start(out=outD[128 * t:128 * (t + 1), :],
                              in_=X[NL][t][:])

    _legalize_sync(nc)
    return nc, dbg_outs


# ---------------------------------------------------------------- host side
def _bf(a):
    return np.asarray(a, dtype=np.float32).astype(ml_dtypes.bfloat16)


def _pcols(vec, ncols):
    """[ncols*128] -> [128, ncols], col j = vec[j*128:(j+1)*128]."""
    return np.ascontiguousarray(
        np.asarray(vec, dtype=np.float32).reshape(ncols, 128).T)


def prepare_inputs(inputs):
    X = np.asarray(inputs["X"], dtype=np.float32)
    S = np.asarray(inputs["S_state"], dtype=np.float32)
    bias = np.asarray(inputs["bias"], dtype=np.float32)
    tok2grid = np.asarray(inputs["tok2grid"]).astype(np.int64)
    Wb = np.asarray(inputs["Wb"], dtype=np.float32)
    Wq = np.asarray(inputs["Wq"], dtype=np.float32)
    Wk = np.asarray(inputs["Wk"], dtype=np.float32)
    Wv = np.asarray(inputs["Wv"], dtype=np.float32)
    Wo_ = np.asarray(inputs["Wo"], dtype=np.float32)
    fc1_W = np.asarray(inputs["fc1_W"], dtype=np.float32)
    fc2_W = np.asarray(inputs["fc2_W"], dtype=np.float32)
    conv_W = np.asarray(inputs["conv_W"], dtype=np.float32)
    bq = np.asarray(inputs["bq"], dtype=np.float32)
    bv = np.asarray(inputs["bv"], dtype=np.float32)

    shared = {}
    # block-diagonal Wb over m-chunks of 8: [(d,mi), (h,mj)] per layer
    wblk = np.zeros([NL, 8, DB, H, 8], np.float32)
    for mi in range(8):
        wblk[:, mi, :, :, mi] = Wb
    shared["wbblk"] = _bf(np.ascontiguousarray(
        wblk.transpose(0, 2, 1, 3, 4).reshape(NL, 128, 64)
        .transpose(1, 0, 2)))
    shared["wqkv"] = _bf(np.concatenate(
        [Wq.transpose(0, 2, 1, 3).reshape(NL, DD, H * DA),
         Wk.transpose(0, 2, 1, 3).reshape(NL, DD, H * DA),
         Wv.transpose(0, 2, 1, 3).reshape(NL, DD, H * DM)], axis=2))
    shared["wo"] = _bf(Wo_)
    shared["fc1w"] = _bf(fc1_W)
    shared["fc2w"] = _bf(fc2_W)
    cw = conv_W.transpose(0, 2, 3, 4, 1).reshape(LC, DD, 9 * DD)
    shared["cww"] = _bf(cw.reshape(LC, 2, 128, 9 * DD))

    lng = np.stack([
        *[_pcols(inputs["ln1_g"][l], 2) for l in range(NL)],
        *[_pcols(inputs["ln2_g"][l], 2) for l in range(NL)],
        *[_pcols(inputs["cln_g"][c], 2) for c in range(LC)]])
    lnbv = np.stack([
        *[_pcols(inputs["ln1_b"][l], 2) for l in range(NL)],
        *[_pcols(inputs["ln2_b"][l], 2) for l in range(NL)],
        *[_pcols(inputs["cln_b"][c], 2) for c in range(LC)]])
    order = [0, 3, 1, 4, 2, 5, 6, 7, 8]
    shared["lngP"] = np.ascontiguousarray(lng[order].transpose(1, 0, 2))
    shared["lnbP"] = np.ascontiguousarray(lnbv[order].transpose(1, 0, 2))
    shared["boP"] = np.ascontiguousarray(np.stack(
        [_pcols(inputs["bo"][l], 2) for l in range(NL)], axis=1))
    shared["b1P"] = np.ascontiguousarray(np.stack(
        [_pcols(inputs["fc1_b"][l], 8) for l in range(NL)], axis=1))
    shared["b2P"] = np.ascontiguousarray(np.stack(
        [_pcols(inputs["fc2_b"][l], 2) for l in range(NL)], axis=1))
    shared["cbP"] = np.ascontiguousarray(np.stack(
        [_pcols(inputs["conv_b"][c], 2) for c in range(NL)], axis=1))
    shared["pP"] = _pcols(inputs["P"], 2)
    bqf = bq.reshape(NL, H * DA)
    shared["bqP"] = np.ascontiguousarray(np.stack(
        [_pcols(bqf[l], 4) for l in range(NL)], axis=1))
    bvf = bv.reshape(NL, H * DM)
    shared["bvP"] = _bf(np.ascontiguousarray(np.stack(
        [_pcols(bvf[l], 4) for l in range(NL)], axis=1)))
    shared["eyeD"] = _bf(np.eye(128, dtype=np.float32))
    obt = np.zeros([16, 16, 64], np.float32)
    for b in range(16):
        obt[b, b, :] = 1.0
    shared["obtD"] = _bf(obt.reshape(16, 1024))

    in_maps = []
    for c in range(NCORES):
        bs = slice(c * BL, (c + 1) * BL)
        m = dict(shared)
        rs2 = 1.0 / math.sqrt(2.0)
        m["xT"] = np.ascontiguousarray(
            X[bs].transpose(2, 0, 1).reshape(DD, R)) * rs2
        m["sT"] = np.ascontiguousarray(
            S[bs].transpose(2, 0, 1).reshape(DD, R)) * rs2
        # [mc, (d,mi), (b,l)]: biasT2[mc, d*8+mi, b*L+l] = bias[b,l,8mc+mi,d]
        bt2 = bias[bs].transpose(2, 3, 0, 1)          # [m, d, b, l]
        bt2 = bt2.reshape(MC, 8, DB, BL, L)           # [mc, mi, d, b, l]
        bt2 = bt2.transpose(0, 2, 1, 3, 4).reshape(MC, 128, BL * L)
        m["biasT2"] = _bf(np.ascontiguousarray(bt2))
        in_maps.append(m)
    return in_maps, tok2grid


_GRAPH_CACHE = {}


def kernel(**inputs):
    in_maps, tok2grid = prepare_inputs(inputs)
    key = tuple(int(v) for v in tok2grid)
    if key not in _GRAPH_CACHE:
        _GRAPH_CACHE[key] = build_graph(tok2grid)[0]
    nc = _GRAPH_CACHE[key]
    res = run_bass_kernel_spmd(nc, in_maps, core_ids=list(range(NCORES)))
    outs = []
    for c in range(NCORES):
        o = res.results[c]["out"].reshape(DD, BL, L).transpose(1, 2, 0)
        outs.append(o)
    return np.ascontiguousarray(np.concatenate(outs, axis=0),
                                dtype=np.float32)


# revision 38
# speedup vs baseline: 1.0126x; 1.0102x over previous
"""AlphaQubit-like decoder on 8 TRN2 NeuronCores (Bass/Tile).

Sharding: data-parallel over batch (16 per core), zero collectives.
Activations are feature-major [d (partitions), (b, l) (free)] so every
matmul contracts on partitions with weights stationary and every bias is
per-partition. bf16 matmul operands; f32 residual stream + PSUM.

Structure (2.21ms -> 1.46ms vs the per-(b,h) baseline):
  - bq folded into Q at QKV PSUM evacuation (per-partition bias), bk
    dropped (softmax-invariant), bv folded via bo_eff = bo + Wo^T bv;
    the (X+S)/sqrt2 prescale is applied host-side.
  - Pairwise-bias projection Bp computed with a block-diagonal lhs over
    m-chunks of 8 so the output lands m-major ([*(h,mj), (b,l)]), then
    stored to DRAM as [NH, m, (b,l)] with fat contiguous rows; attention
    loads one [120, 1920] head tile per (l,h) with 3.8KB descriptors
    (the v1 thin-row transpose loads burned ~700us of DMA-queue time).
  - Attention batched per (l,h) in a 3-stage head pipeline: S^T in
    [120,480] PSUM quarters (+Bp via identity matmul), one exp per
    quarter; row-sums computed TRANSPOSED ([120(l),16(b)] via E-as-lhs
    N=1 matmuls) so the DVE reciprocal (6.3ns/free-elem!) runs on 16
    free elems instead of 46080; orientation restored by an
    identity-rhs transpose matmul + delta-broadcast matmuls; one DVE
    multiply per quarter normalizes A@V.
  - LayerNorm rs = exp(-0.5*ln(var+eps)) on the Activation engine (ln
    and exp share one act table with attention's exp; no DVE
    reciprocal); squares on DVE in bf16; mean/rs broadcast by K=1
    matmuls; chunk-staggered emission keeps the scalar rsqrt tail off
    the PE queue's head.
  - FFN gate: (fc1g + b) * gelu(fc1a + b) via one scalar_tensor_tensor.
  - Conv LN writer uses 3D-AP activations; conv matmuls in 3-batch
    N=432 chunks; scatter/gather and pad-zeroing split across engines
    and hoisted off the conv-entry critical path.
  - Bulk DMAs rotate over gpsimd/scalar queues; sync is reserved for
    bias-tile and Bp head loads so they never queue behind stores.
"""

import contextlib
import itertools
import math

import numpy as np
import ml_dtypes

import concourse.bass as bass
import concourse.mybir as mybir
import concourse.tile as tile
from concourse.bass_utils import run_bass_kernel_spmd

F32 = mybir.dt.float32
BF16 = mybir.dt.bfloat16
AX = mybir.AluOpType
AF = mybir.ActivationFunctionType

B, L, DD = 128, 120, 256
DA, DM, H, DB = 64, 64, 8, 16
NL, LC = 3, 3
DILS = (1, 2, 4)
G = 12
NCORES = 8
BL = B // NCORES            # 16 batches per core
R = BL * L                  # 1920 seq cols per core
CELLS = G * G               # 144
RG = BL * CELLS             # 2304 grid cols per core
NH = NL * H                 # 24 (layer,head) rows
SP = 20                     # padded grid side (pad=4 fits dilation<=4)
PADC = SP * SP
CHUNK = 480                 # seq chunk (4 batches)
NCH = R // CHUNK            # 4 seq chunks
GCH = 288                   # grid chunk (2 batches)
MC = 15                     # m-chunks of 8 in the Bp projection
EPS = 1e-5
SCALE = 1.0 / math.sqrt(DA)


def _legalize_sync(nc):
    """This walrus build caps per-instruction semaphore waits (drain: 0,
    matmul/ldweights: 1, others: 2); move excess waits onto NoOps."""
    caps = {"InstDrain": 0}
    for f in nc.m.functions:
        for bb in f.blocks:
            new_insts = []
            for inst in bb.instructions:
                si = getattr(inst, "sync_info", None)
                cap = caps.get(type(inst).__name__, 1)
                if si is not None and si.on_wait and len(si.on_wait) > cap:
                    waits = list(si.on_wait)
                    keep = waits[len(waits) - cap:] if cap else []
                    for i, w in enumerate(waits[: len(waits) - cap]):
                        new_insts.append(mybir.InstNoOp(
                            name=inst.name + f"-ws{i}", engine=inst.engine,
                            ins=[], outs=[],
                            sync_info=mybir.SyncInfo(on_wait=[w],
                                                     on_update=[])))
                    si.on_wait = keep
                new_insts.append(inst)
            bb.instructions = new_insts


def build_graph(tok2grid, debug_taps=()):
    dbg = set(debug_taps)
    nc = bass.Bass()

    xT = nc.dram_tensor("xT", [DD, R], F32, kind="ExternalInput")
    sT = nc.dram_tensor("sT", [DD, R], F32, kind="ExternalInput")
    biasT2 = nc.dram_tensor("biasT2", [MC, 128, R], BF16,
                            kind="ExternalInput")
    wbblk = nc.dram_tensor("wbblk", [128, NL, 64], BF16,
                           kind="ExternalInput")
    wqkv = nc.dram_tensor("wqkv", [NL, DD, 1536], BF16, kind="ExternalInput")
    wo = nc.dram_tensor("wo", [NL, H * DM, DD], BF16, kind="ExternalInput")
    fc1w = nc.dram_tensor("fc1w", [NL, DD, 1024], BF16, kind="ExternalInput")
    fc2w = nc.dram_tensor("fc2w", [NL, 512, DD], BF16, kind="ExternalInput")
    cww = nc.dram_tensor("cww", [LC, 2, 128, 9 * DD], BF16,
                         kind="ExternalInput")
    lngP = nc.dram_tensor("lngP", [128, 9, 2], F32, kind="ExternalInput")
    lnbP = nc.dram_tensor("lnbP", [128, 9, 2], F32, kind="ExternalInput")
    boP = nc.dram_tensor("boP", [128, NL, 2], F32, kind="ExternalInput")
    b1P = nc.dram_tensor("b1P", [128, NL, 8], F32, kind="ExternalInput")
    b2P = nc.dram_tensor("b2P", [128, NL, 2], F32, kind="ExternalInput")
    cbP = nc.dram_tensor("cbP", [128, LC, 2], F32, kind="ExternalInput")
    pP = nc.dram_tensor("pP", [128, 2], F32, kind="ExternalInput")
    bqP = nc.dram_tensor("bqP", [128, NL, 4], F32, kind="ExternalInput")
    bvP = nc.dram_tensor("bvP", [128, NL, 4], BF16, kind="ExternalInput")
    eyeD = nc.dram_tensor("eyeD", [128, 128], BF16, kind="ExternalInput")
    obtD = nc.dram_tensor("obtD", [16, 1024], BF16, kind="ExternalInput")
    outD = nc.dram_tensor("out", [DD, R], F32, kind="ExternalOutput")

    dbg_outs = {}

    def dbg_tensor(name, shape):
        dbg_outs[name] = nc.dram_tensor(name, shape, F32,
                                        kind="ExternalOutput")
        return dbg_outs[name]

    # token<->grid runs (host-known permutation baked into APs)
    t2g = [int(v) for v in tok2grid]
    assert len(set(t2g)) == L and all(0 <= v < CELLS for v in t2g)
    runs = []
    s = 0
    for i in range(1, L + 1):
        if i == L or t2g[i] != t2g[i - 1] + 1:
            runs.append((s, t2g[s], i - s))
            s = i
    unmapped = sorted(set(range(CELLS)) - set(t2g))
    uruns = []
    if unmapped:
        s = 0
        for i in range(1, len(unmapped) + 1):
            if i == len(unmapped) or unmapped[i] != unmapped[i - 1] + 1:
                uruns.append((unmapped[s], i - s))
                s = i

    uid = [0]

    def pool(lctx, name, bufs, space="SBUF"):
        uid[0] += 1
        return lctx.enter_context(
            tc.tile_pool(name=f"{name}_{uid[0]}", bufs=bufs, space=space))

    # rotate bulk DMAs across engine queues
    qrr = itertools.cycle(["gpsimd", "scalar"])

    def dma(out, in_):
        getattr(nc, next(qrr)).dma_start(out=out, in_=in_)

    with tile.TileContext(nc) as tc, contextlib.ExitStack() as ctx:
        wp = ctx.enter_context(tc.tile_pool(name="weights", bufs=1))
        xp = ctx.enter_context(tc.tile_pool(name="xres", bufs=2))
        dram = ctx.enter_context(tc.tile_pool(name="dram", bufs=1,
                                              space="DRAM"))

        # ---------------- weights/constants to SBUF
        w_qkv = [[wp.tile([128, 1536], BF16, tag=f"wqkv{l}{k}",
                          name=f"wqkv{l}{k}") for k in range(2)]
                 for l in range(NL)]
        w_o = [[wp.tile([128, DD], BF16, tag=f"wo{l}{k}", name=f"wo{l}{k}")
                for k in range(4)] for l in range(NL)]
        w_f1 = [[wp.tile([128, 1024], BF16, tag=f"f1{l}{k}", name=f"f1{l}{k}")
                 for k in range(2)] for l in range(NL)]
        w_f2 = [[wp.tile([128, DD], BF16, tag=f"f2{l}{k}", name=f"f2{l}{k}")
                 for k in range(4)] for l in range(NL)]
        w_cv = [[wp.tile([128, 9 * DD], BF16, tag=f"cv{c}{k}",
                         name=f"cv{c}{k}") for k in range(2)]
                for c in range(LC)]
        for k in range(2):
            dma(w_qkv[0][k][:], wqkv[0, 128 * k:128 * (k + 1), :])
        for l in range(NL):
            for k in range(4):
                dma(w_o[l][k][:], wo[l, 128 * k:128 * (k + 1), :])

        def load_late_weights():
            for l in range(NL):
                for k in range(2):
                    if l > 0:
                        dma(w_qkv[l][k][:],
                            wqkv[l, 128 * k:128 * (k + 1), :])
                    dma(w_f1[l][k][:], fc1w[l, 128 * k:128 * (k + 1), :])
                for k in range(4):
                    dma(w_f2[l][k][:], fc2w[l, 128 * k:128 * (k + 1), :])
            for c in range(LC):
                for k in range(2):
                    dma(w_cv[c][k][:], cww[c, k, :, :])

        lng = wp.tile([128, 9, 2], F32)
        dma(lng[:], lngP[:])
        lnb = wp.tile([128, 9, 2], F32)
        dma(lnb[:], lnbP[:])
        bo_s = wp.tile([128, NL, 2], F32)
        dma(bo_s[:], boP[:])
        b1_s = wp.tile([128, NL, 8], F32)
        dma(b1_s[:], b1P[:])
        b2_s = wp.tile([128, NL, 2], F32)
        dma(b2_s[:], b2P[:])
        cb_s = wp.tile([128, LC, 2], F32)
        dma(cb_s[:], cbP[:])
        p_s = wp.tile([128, 2], F32)
        dma(p_s[:], pP[:])
        bq_s = wp.tile([128, NL, 4], F32)
        dma(bq_s[:], bqP[:])
        bv_s = wp.tile([128, NL, 4], BF16)
        dma(bv_s[:], bvP[:])
        eye_s = wp.tile([128, 128], BF16)
        dma(eye_s[:], eyeD[:])
        obt_s = wp.tile([16, 1024], BF16)
        dma(obt_s[:], obtD[:])
        wbb_s = wp.tile([128, NL, 64], BF16)
        dma(wbb_s[:], wbblk[:])

        ones_rowb = wp.tile([1, 128], BF16)
        nc.vector.memset(ones_rowb[:], 1.0)
        invd_col = wp.tile([128, 1], BF16)
        nc.vector.memset(invd_col[:], 1.0 / DD)
        onec = wp.tile([128, 1], BF16)
        nc.vector.memset(onec[:], 1.0)
        eps_col = wp.tile([128, 1], F32)
        nc.vector.memset(eps_col[:], EPS)
        zero_col = wp.tile([128, 1], F32)
        nc.vector.memset(zero_col[:], 0.0)

        # fold bv into bo: bo_eff = bo + Wo^T bv
        bo_eff = wp.tile([128, NL, 2], F32)
        with tc.tile_pool(name="prep_ps", bufs=2, space="PSUM") as pps:
            for l in range(NL):
                d_ps = pps.tile([128, 2], F32, tag="dps", name="dps")
                for ot in range(2):
                    for k in range(4):
                        nc.tensor.matmul(
                            d_ps[:, ot:ot + 1],
                            w_o[l][k][:, 128 * ot:128 * (ot + 1)],
                            bv_s[:, l, k:k + 1],
                            start=(k == 0), stop=(k == 3))
                nc.vector.tensor_add(out=bo_eff[:, l], in0=d_ps[:],
                                     in1=bo_s[:, l])

        # ---------------- residual init: x = X + S (host pre-scaled 1/sqrt2)
        X = [[xp.tile([128, R], F32, tag=f"X{t}", name=f"X{t}")
              for t in range(2)]]
        with contextlib.ExitStack() as lctx:
            ipool = pool(lctx, "init", 2)
            for t in range(2):
                xi = ipool.tile([128, R], F32, tag="xi", name="xi")
                si = ipool.tile([128, R], F32, tag="si", name="si")
                nc.sync.dma_start(out=xi[:], in_=xT[128 * t:128 * (t + 1), :])
                nc.sync.dma_start(out=si[:], in_=sT[128 * t:128 * (t + 1), :])
                nc.vector.tensor_add(out=X[0][t][:], in0=xi[:], in1=si[:])
        if "x0" in dbg:
            d = dbg_tensor("d_x0", [DD, R])
            for t in range(2):
                nc.sync.dma_start(out=d[128 * t:128 * (t + 1), :],
                                  in_=X[0][t][:])

        # ---------------- layernorm (feature-major, chunk-staggered)
        def layernorm(lctx, li, src, n_chunks, ccols, writer):
            """src: 2 f32 tiles [128, n_chunks*ccols]. writer(t, c, xc, rsb)
            must emit (xc * g) * rs + b into its target; xc f32 SBUF
            [128, ccols], rsb f32 PSUM [128, ccols] broadcast of rs.
            Stats matmuls of chunk c+1 are emitted before the broadcast of
            chunk c so the PE queue never stalls on the scalar rsqrt tail."""
            sps = pool(lctx, "ln_st", 2, "PSUM")
            bps = pool(lctx, "ln_bc", 2, "PSUM")
            sp = pool(lctx, "ln_sb", 2)

            def stats(c):
                cs = slice(c * ccols, (c + 1) * ccols)
                xb = [sp.tile([128, 2, ccols], BF16, tag=f"xb{t}",
                              name=f"xb{t}") for t in range(2)]
                for t in range(2):
                    nc.gpsimd.dma_start(out=xb[t][:, 0], in_=src[t][:, cs])
                for t in range(2):   # squares on DVE (bf16 fast mode)
                    nc.vector.tensor_mul(out=xb[t][:, 1], in0=xb[t][:, 0],
                                         in1=xb[t][:, 0])
                st = sps.tile([1, 2, 512], F32, tag="st", name="st")
                for t in range(2):
                    nc.tensor.matmul(st[:, 0, :ccols], invd_col[:],
                                     xb[t][:, 0], start=(t == 0),
                                     stop=(t == 1))
                for t in range(2):
                    nc.tensor.matmul(st[:, 1, :ccols], invd_col[:],
                                     xb[t][:, 1], start=(t == 0),
                                     stop=(t == 1))
                return st

            def tail(c, st):
                cs = slice(c * ccols, (c + 1) * ccols)
                mrs = sp.tile([1, 2, ccols], BF16, tag="mrs", name="mrs")
                nc.scalar.copy(mrs[:, 0], st[:, 0, :ccols])
                sqm = sp.tile([1, ccols], F32, tag="sqm", name="sqm")
                nc.scalar.square(sqm[:], mrs[:, 0])
                lnv = sp.tile([1, ccols], F32, tag="lnv", name="lnv")
                nc.vector.tensor_sub(out=lnv[:], in0=st[:, 1, :ccols],
                                     in1=sqm[:])
                # rs = (var+eps)^-1/2 = exp(-0.5*ln(var+eps))
                nc.scalar.activation(lnv[:], lnv[:], AF.Ln,
                                     bias=eps_col[0:1, :], scale=1.0)
                nc.scalar.activation(mrs[:, 1], lnv[:], AF.Exp,
                                     bias=zero_col[0:1, :], scale=-0.5)
                mb = bps.tile([128, ccols], F32, tag="mb", name="mb")
                nc.tensor.matmul(mb[:], ones_rowb[:], mrs[:, 0],
                                 start=True, stop=True)
                rsb = bps.tile([128, ccols], F32, tag="rsb", name="rsb")
                nc.tensor.matmul(rsb[:], ones_rowb[:], mrs[:, 1],
                                 start=True, stop=True)
                for t in range(2):
                    xc = sp.tile([128, ccols], BF16, tag=f"xc{t}",
                                 name=f"xc{t}")
                    nc.vector.tensor_sub(out=xc[:], in0=src[t][:, cs],
                                         in1=mb[:])
                    writer(t, c, xc, rsb)

            prev = None
            for c in range(n_chunks):
                st = stats(c)
                if prev is not None:
                    tail(*prev)
                prev = (c, st)
            tail(*prev)

        # ---------------- Bp projection: m-major blocks -> DRAM
        BpTd = dram.tile([NH, 120, R], BF16)

        def emit_bp_projection():
            with contextlib.ExitStack() as lctx:
                btp = pool(lctx, "bp_in", 3)
                bpp = pool(lctx, "bp_ps", 8, "PSUM")
                bst = pool(lctx, "bp_st", 3)
                for mc in range(MC):
                    bt = btp.tile([128, R], BF16, tag="bt", name="bt")
                    nc.sync.dma_start(out=bt[:], in_=biasT2[mc])
                    for l in range(NL):
                        stage = bst.tile([64, R], BF16, tag="bstg",
                                         name="bstg")
                        for q in range(NCH):
                            qs = slice(q * CHUNK, (q + 1) * CHUNK)
                            ps = bpp.tile([64, CHUNK], F32, tag="bpp",
                                          name="bpp")
                            nc.tensor.matmul(ps[:], wbb_s[:, l, :],
                                             bt[:, qs],
                                             start=True, stop=True)
                            if (mc + l + q) % 2 == 0:
                                nc.scalar.copy(stage[:, qs], ps[:])
                            else:
                                nc.vector.tensor_copy(out=stage[:, qs],
                                                      in_=ps[:])
                        for h in range(H):
                            dma(BpTd[8 * l + h, 8 * mc:8 * (mc + 1), :],
                                stage[8 * h:8 * (h + 1), :])

        load_late_weights()

        if dbg:   # debug taps need the projection data eagerly
            emit_bp_projection()
        if "bp" in dbg:
            d_bp = nc.dram_tensor("d_bp", [NH, 120, R], BF16,
                                  kind="ExternalOutput")
            dbg_outs["d_bp"] = d_bp
            nc.sync.dma_start(out=d_bp[:], in_=BpTd[:])

        # ---------------- layers
        for l in range(NL):
            Xc = X[l]
            with contextlib.ExitStack() as lp:
                op_ = pool(lp, f"op{l}", 1)    # O^T: lives till Wo
                O = [op_.tile([128, R], BF16, tag=f"o{ot}", name=f"o{ot}")
                     for ot in range(4)]
                qa = contextlib.ExitStack()
                lp.enter_context(qa)
                qv = pool(qa, f"qv{l}", 1)   # qk/V live till attn end
                qk = [qv.tile([128, R], BF16, tag=f"qk{ot}",
                              name=f"qk{ot}") for ot in range(8)]
                V = [qv.tile([120, 512], BF16, tag=f"v{b}",
                             name=f"v{b}") for b in range(BL)]
                # ---- LN1 + QKV (Xn scoped)
                with contextlib.ExitStack() as ph:
                    xnp = pool(ph, f"xn{l}", 1)
                    Xn = [xnp.tile([128, R], BF16, tag=f"xn{t}",
                                   name=f"xn{t}") for t in range(2)]

                    def w_ln1(t, c, xc, rsb, l=l, Xn=Xn):
                        tgt = Xn[t][:, c * CHUNK:(c + 1) * CHUNK]
                        nc.vector.tensor_mul(out=xc[:], in0=xc[:],
                                             in1=rsb[:])
                        nc.vector.tensor_scalar(
                            out=tgt, in0=xc[:],
                            scalar1=lng[:, 2 * l, t:t + 1],
                            scalar2=lnb[:, 2 * l, t:t + 1],
                            op0=AX.mult, op1=AX.add)

                    with contextlib.ExitStack() as lnx:
                        layernorm(lnx, 2 * l, Xc, NCH, CHUNK, w_ln1)
                    if l == 0 and "ln1" in dbg:
                        d = dbg_tensor("d_ln1", [DD, R])
                        for t in range(2):
                            nc.gpsimd.dma_start(
                                out=d[128 * t:128 * (t + 1), :],
                                in_=Xn[t][:])

                    qps = pool(ph, f"qkv_ps{l}", 2, "PSUM")
                    for c in range(NCH):
                        cs = slice(c * CHUNK, (c + 1) * CHUNK)
                        for ot in range(8):
                            ps = qps.tile([128, CHUNK], F32, tag="qkp",
                                          name="qkp", bufs=5)
                            for k in range(2):
                                nc.tensor.matmul(
                                    ps[:],
                                    w_qkv[l][k][:, 128 * ot:128 * (ot + 1)],
                                    Xn[k][:, cs], start=(k == 0),
                                    stop=(k == 1))
                            if ot < 4:   # Q tiles: fold bq in the evac
                                nc.scalar.activation(
                                    qk[ot][:, cs], ps[:], AF.Identity,
                                    bias=bq_s[:, l, ot:ot + 1], scale=1.0)
                            else:
                                nc.vector.tensor_copy(out=qk[ot][:, cs],
                                                      in_=ps[:])
                    for b in range(BL):
                        bs = slice(b * L, (b + 1) * L)
                        ps = qps.tile([120, 512], F32, tag="vp", name="vp")
                        for k in range(2):
                            nc.tensor.matmul(
                                ps[:], Xn[k][:, bs],
                                w_qkv[l][k][:, 1024:1536],
                                start=(k == 0), stop=(k == 1))
                        if b % 2 == 0:
                            nc.scalar.copy(V[b][:], ps[:])
                        else:
                            nc.vector.tensor_copy(out=V[b][:], in_=ps[:])

                if l == 0 and "qkv" in dbg:
                    dq = dbg_tensor("d_q0", [512, R])
                    dk = dbg_tensor("d_k0", [512, R])
                    for ot in range(4):
                        nc.gpsimd.dma_start(
                            out=dq[128 * ot:128 * (ot + 1), :],
                            in_=qk[ot][:])
                        nc.gpsimd.dma_start(
                            out=dk[128 * ot:128 * (ot + 1), :],
                            in_=qk[4 + ot][:])
                    dv = dbg_tensor("d_v0", [BL * 120, 512])
                    for b in range(BL):
                        nc.gpsimd.dma_start(
                            out=dv[120 * b:120 * (b + 1), :], in_=V[b][:])

                if l == 0 and not dbg:
                    emit_bp_projection()

                # ---- attention (batched per head, 3-stage head pipeline)
                with contextlib.ExitStack() as ph:
                    bpl = pool(ph, f"bpl{l}", 3)
                    epl = pool(ph, f"ep{l}", 3)
                    smp = pool(ph, f"sm{l}", 2)
                    spsp = pool(ph, f"attS{l}", 2, "PSUM")
                    rtp = pool(ph, f"attRT{l}", 1, "PSUM")
                    rbtp = pool(ph, f"attRB{l}", 1, "PSUM")
                    rbp = pool(ph, f"attRb{l}", 2, "PSUM")
                    ops_ = pool(ph, f"attO{l}", 2, "PSUM")
                    st_e = {}
                    st_rbt = {}

                    def stage_a(h, l=l):
                        qt, qb = h // 2, (h % 2) * 64
                        Bph = bpl.tile([120, R], BF16, tag="bph",
                                       name="bph")
                        nc.sync.dma_start(out=Bph[:], in_=BpTd[8 * l + h])
                        # exp(s+Bp) = exp(s) * exp(Bp): precompute the bias
                        # factor once per head, multiply on DVE in bf16
                        EB = bpl.tile([120, R], BF16, tag="eb", name="eb")
                        nc.scalar.activation(EB[:], Bph[:], AF.Exp,
                                             bias=zero_col[:120, :],
                                             scale=SCALE)
                        E = epl.tile([120, R], BF16, tag="E", name="E")
                        for q in range(NCH):
                            qs = slice(q * CHUNK, (q + 1) * CHUNK)
                            sps_t = spsp.tile([120, CHUNK], F32, tag="sps",
                                              name="sps")
                            for bi in range(4):
                                b = q * 4 + bi
                                bs = slice(b * L, (b + 1) * L)
                                nc.tensor.matmul(
                                    sps_t[:, bi * L:(bi + 1) * L],
                                    qk[4 + qt][qb:qb + 64, bs],
                                    qk[qt][qb:qb + 64, bs],
                                    start=True, stop=True)
                            nc.scalar.activation(
                                E[:, qs], sps_t[:], AF.Exp,
                                bias=zero_col[:120, :], scale=SCALE)
                            nc.vector.tensor_mul(out=E[:, qs], in0=E[:, qs],
                                                 in1=EB[:, qs])
                        st_e[h] = E

                    def stage_b(h):
                        E = st_e[h]
                        rT = rtp.tile([120, 16], F32, tag="rT", name="rT")
                        for b in range(BL):
                            nc.tensor.matmul(
                                rT[:, b:b + 1], E[:, b * L:(b + 1) * L],
                                onec[:120, :], start=True, stop=True)
                        rTi = smp.tile([120, 16], BF16, tag="rTi",
                                       name="rTi")
                        with nc.allow_low_precision(
                                reason="softmax 1/Z in bf16 feeds bf16 "
                                       "matmul broadcast"):
                            nc.vector.reciprocal(out=rTi[:], in_=rT[:])
                        rbt_ps = rbtp.tile([16, 120], F32, tag="rbt",
                                           name="rbt")
                        nc.tensor.matmul(rbt_ps[:], rTi[:],
                                         eye_s[:120, :120],
                                         start=True, stop=True)
                        rbt = smp.tile([16, 120], BF16, tag="rbs",
                                       name="rbs")
                        nc.scalar.copy(rbt[:], rbt_ps[:])
                        st_rbt[h] = rbt

                    def stage_c(h):
                        qt, qb = h // 2, (h % 2) * 64
                        E, rbt = st_e.pop(h), st_rbt.pop(h)
                        for q in range(NCH):
                            qs = slice(q * CHUNK, (q + 1) * CHUNK)
                            rb_t = rbp.tile([64, CHUNK], F32, tag="rbq",
                                            name="rbq")
                            o_t = ops_.tile([64, CHUNK], F32, tag="opq",
                                            name="opq")
                            for bi in range(4):
                                b = q * 4 + bi
                                nc.tensor.matmul(
                                    rb_t[:, bi * L:(bi + 1) * L],
                                    obt_s[:, 64 * b:64 * (b + 1)], rbt[:],
                                    start=True, stop=True)
                                nc.tensor.matmul(
                                    o_t[:, bi * L:(bi + 1) * L],
                                    V[b][:, 64 * h:64 * (h + 1)],
                                    E[:, b * L:(b + 1) * L],
                                    start=True, stop=True)
                            rb_sb = smp.tile([64, CHUNK], BF16, tag="rbb",
                                             name="rbb")
                            if (h + q) % 2 == 0:
                                nc.scalar.copy(rb_sb[:], rb_t[:])
                            else:
                                nc.vector.tensor_copy(out=rb_sb[:],
                                                      in_=rb_t[:])
                            nc.vector.tensor_mul(
                                out=O[qt][qb:qb + 64, qs], in0=o_t[:],
                                in1=rb_sb[:])

                    for h in range(H):
                        stage_a(h)
                        if h >= 1:
                            stage_b(h - 1)
                        if h >= 2:
                            stage_c(h - 2)
                    stage_b(H - 1)
                    stage_c(H - 2)
                    stage_c(H - 1)
                qa.close()   # free qk/V
                if l == 0 and "att" in dbg:
                    do = dbg_tensor("d_att0", [512, R])
                    for ot in range(4):
                        nc.gpsimd.dma_start(
                            out=do[128 * ot:128 * (ot + 1), :], in_=O[ot][:])

                # ---- Wo + residual
                Xa = [xp.tile([128, R], F32, tag=f"X{t}", name=f"X{t}")
                      for t in range(2)]
                with contextlib.ExitStack() as ph:
                    wps = pool(ph, f"wo_ps{l}", 6, "PSUM")
                    for c in range(NCH):
                        cs = slice(c * CHUNK, (c + 1) * CHUNK)
                        for ot in range(2):
                            ps = wps.tile([128, CHUNK], F32, tag="wop",
                                          name="wop")
                            for k in range(4):
                                nc.tensor.matmul(
                                    ps[:],
                                    w_o[l][k][:, 128 * ot:128 * (ot + 1)],
                                    O[k][:, cs], start=(k == 0),
                                    stop=(k == 3))
                            nc.vector.scalar_tensor_tensor(
                                out=Xa[ot][:, cs], in0=ps[:],
                                scalar=bo_eff[:, l, ot:ot + 1],
                                in1=Xc[ot][:, cs], op0=AX.add, op1=AX.add)
                if l == 0 and "wo" in dbg:
                    d = dbg_tensor("d_wo0", [DD, R])
                    for t in range(2):
                        nc.sync.dma_start(out=d[128 * t:128 * (t + 1), :],
                                          in_=Xa[t][:])

                # ---- LN2 + gated FFN (pad grid zeroed here: fills queue
                # slack long before the conv block reads it)
                gp = pool(lp, f"grid{l}", 1)
                pad = [gp.tile([128, BL * PADC], BF16, tag=f"pad{t}",
                               name=f"pad{t}") for t in range(2)]
                nc.vector.memset(pad[0][:], 0.0)
                nc.gpsimd.memset(pad[1][:], 0.0)
                Xf = [xp.tile([128, R], F32, tag=f"X{t}", name=f"X{t}")
                      for t in range(2)]
                with contextlib.ExitStack() as ph:
                    xnp = pool(ph, f"xn2{l}", 1)
                    Xn2 = [xnp.tile([128, R], BF16, tag=f"xn2{t}",
                                    name=f"xn2{t}") for t in range(2)]

                    def w_ln2(t, c, xc, rsb, l=l, Xn2=Xn2):
                        tgt = Xn2[t][:, c * CHUNK:(c + 1) * CHUNK]
                        nc.vector.tensor_mul(out=xc[:], in0=xc[:],
                                             in1=rsb[:])
                        nc.scalar.activation(
                            tgt, xc[:], AF.Identity,
                            bias=lnb[:, 2 * l + 1, t:t + 1],
                            scale=lng[:, 2 * l + 1, t:t + 1])

                    with contextlib.ExitStack() as lnx:
                        layernorm(lnx, 2 * l + 1, Xa, NCH, CHUNK, w_ln2)

                    hp_ = pool(ph, f"hh{l}", 1)
                    Hh = [hp_.tile([128, R], BF16, tag=f"h{ot}",
                                   name=f"h{ot}") for ot in range(4)]
                    fps = pool(ph, f"f1ps{l}", 3, "PSUM")
                    fsb = pool(ph, f"f1sb{l}", 3)
                    for c in range(NCH):
                        cs = slice(c * CHUNK, (c + 1) * CHUNK)
                        for ot in range(4):
                            ps = fps.tile([128, CHUNK], F32, tag="f1a",
                                          name="f1a")
                            for k in range(2):
                                nc.tensor.matmul(
                                    ps[:],
                                    w_f1[l][k][:, 128 * ot:128 * (ot + 1)],
                                    Xn2[k][:, cs], start=(k == 0),
                                    stop=(k == 1))
                            ga = fsb.tile([128, CHUNK], BF16, tag="ga",
                                          name="ga")
                            nc.scalar.activation(
                                ga[:], ps[:], AF.Gelu,
                                bias=b1_s[:, l, ot:ot + 1], scale=1.0)
                            ps2 = fps.tile([128, CHUNK], F32, tag="f1g",
                                           name="f1g")
                            for k in range(2):
                                nc.tensor.matmul(
                                    ps2[:],
                                    w_f1[l][k][:, 512 + 128 * ot:
                                               512 + 128 * (ot + 1)],
                                    Xn2[k][:, cs], start=(k == 0),
                                    stop=(k == 1))
                            nc.vector.scalar_tensor_tensor(
                                out=Hh[ot][:, cs], in0=ps2[:],
                                scalar=b1_s[:, l, 4 + ot:5 + ot],
                                in1=ga[:], op0=AX.add, op1=AX.mult)
                    f2ps = pool(ph, f"f2ps{l}", 2, "PSUM")
                    for c in range(NCH):
                        cs = slice(c * CHUNK, (c + 1) * CHUNK)
                        for ot in range(2):
                            ps = f2ps.tile([128, CHUNK], F32, tag="f2p",
                                           name="f2p")
                            for k in range(4):
                                nc.tensor.matmul(
                                    ps[:],
                                    w_f2[l][k][:, 128 * ot:128 * (ot + 1)],
                                    Hh[k][:, cs], start=(k == 0),
                                    stop=(k == 3))
                            nc.vector.scalar_tensor_tensor(
                                out=Xf[ot][:, cs], in0=ps[:],
                                scalar=b2_s[:, l, ot:ot + 1],
                                in1=Xa[ot][:, cs], op0=AX.add, op1=AX.add)
                if l == 0 and "ffn" in dbg:
                    d = dbg_tensor("d_ffn0", [DD, R])
                    for t in range(2):
                        nc.sync.dma_start(out=d[128 * t:128 * (t + 1), :],
                                          in_=Xf[t][:])

                # ---- conv block (shared params)
                with contextlib.ExitStack() as ph:
                    Gt = [gp.tile([128, RG], F32, tag=f"g{t}", name=f"g{t}")
                          for t in range(2)]
                    for t in range(2):
                        xv = Xf[t][:].rearrange("p (b l) -> p b l", b=BL)
                        gv = Gt[t][:].rearrange("p (b c) -> p b c", b=BL)
                        for (ts_, cs_, ln_) in runs:
                            if t == 0:
                                nc.scalar.copy(gv[:, :, cs_:cs_ + ln_],
                                               xv[:, :, ts_:ts_ + ln_])
                            else:
                                nc.vector.tensor_copy(
                                    out=gv[:, :, cs_:cs_ + ln_],
                                    in_=xv[:, :, ts_:ts_ + ln_])
                        for (cs_, ln_) in uruns:
                            nc.scalar.activation(
                                gv[:, :, cs_:cs_ + ln_],
                                xv[:, :, 0:ln_], AF.Identity,
                                bias=p_s[:, t:t + 1], scale=0.0)

                    for cv in range(LC):
                        dil = DILS[cv]

                        def w_cln(t, c, xc, rsb, cv=cv):
                            b0 = c * 2
                            pv = pad[t][:].rearrange(
                                "p (b y x) -> p b y x", y=SP, x=SP)
                            nc.vector.tensor_mul(out=xc[:], in0=xc[:],
                                                 in1=rsb[:])
                            xcv = xc[:].rearrange(
                                "p (b y x) -> p b y x", y=G, x=G)
                            nc.vector.tensor_scalar(
                                out=pv[:, b0:b0 + 2, 4:16, 4:16],
                                in0=xcv[:],
                                scalar1=lng[:, 6 + cv, t:t + 1],
                                scalar2=lnb[:, 6 + cv, t:t + 1],
                                op0=AX.mult, op1=AX.add)

                        with contextlib.ExitStack() as lnx:
                            layernorm(lnx, 6 + cv, Gt, RG // GCH, GCH,
                                      w_cln)
                        with contextlib.ExitStack() as cvx:
                            cps = pool(cvx, f"cvps{l}{cv}", 6, "PSUM")
                            csb = pool(cvx, f"cvsb{l}{cv}", 4)
                            chunks = [(0, 3), (3, 3), (6, 3),
                                      (9, 3), (12, 3), (15, 1)]
                            for grp in range(2):
                                gch = chunks[3 * grp:3 * (grp + 1)]
                                pss = {}
                                for ot in range(2):
                                    for ci in range(3):
                                        pss[(ot, ci)] = cps.tile(
                                            [128, 432], F32, tag="cvp",
                                            name="cvp")
                                for ot in range(2):
                                    for tap in range(9):
                                        ky, kx = tap // 3, tap % 3
                                        y0 = 4 + (ky - 1) * dil
                                        x0 = 4 + (kx - 1) * dil
                                        for k in range(2):
                                            lhs = w_cv[cv][k][
                                                :, tap * DD + 128 * ot:
                                                tap * DD + 128 * (ot + 1)]
                                            for ci, (b0, nb) in \
                                                    enumerate(gch):
                                                pv = pad[k][:].rearrange(
                                                    "p (b y x) -> p b y x",
                                                    y=SP, x=SP)
                                                rhs = pv[:, b0:b0 + nb,
                                                         y0:y0 + G,
                                                         x0:x0 + G]
                                                nc.tensor.matmul(
                                                    pss[(ot, ci)]
                                                    [:, :nb * CELLS],
                                                    lhs, rhs,
                                                    start=(tap == 0 and
                                                           k == 0),
                                                    stop=(tap == 8 and
                                                          k == 1))
                                for ot in range(2):
                                    for ci, (b0, nb) in enumerate(gch):
                                        gs = slice(b0 * CELLS,
                                                   (b0 + nb) * CELLS)
                                        nn_ = nb * CELLS
                                        tt = csb.tile([128, 432], BF16,
                                                      tag="cvt", name="cvt")
                                        nc.scalar.activation(
                                            tt[:, :nn_],
                                            pss[(ot, ci)][:, :nn_],
                                            AF.Gelu,
                                            bias=cb_s[:, cv, ot:ot + 1],
                                            scale=1.0)
                                        nc.vector.tensor_add(
                                            out=Gt[ot][:, gs],
                                            in0=Gt[ot][:, gs],
                                            in1=tt[:, :nn_])

                    # gather tokens back
                    Xo = [xp.tile([128, R], F32, tag=f"X{t}", name=f"X{t}")
                          for t in range(2)]
                    for t in range(2):
                        xv = Xo[t][:].rearrange("p (b l) -> p b l", b=BL)
                        gv = Gt[t][:].rearrange("p (b c) -> p b c", b=BL)
                        for (ts_, cs_, ln_) in runs:
                            if t == 0:
                                nc.scalar.copy(xv[:, :, ts_:ts_ + ln_],
                                               gv[:, :, cs_:cs_ + ln_])
                            else:
                                nc.vector.tensor_copy(
                                    out=xv[:, :, ts_:ts_ + ln_],
                                    in_=gv[:, :, cs_:cs_ + ln_])
                    X.append(Xo)
                if l == 0 and "conv" in dbg:
                    d = dbg_tensor("d_conv0", [DD, R])
                    for t in range(2):
                        nc.sync.dma_start(out=d[128 * t:128 * (t + 1), :],
                                          in_=X[l + 1][t][:])

        for t in range(2):
            nc.sync.dma_start(out=outD[128 * t:128 * (t + 1), :],
                              in_=X[NL][t][:])

    _legalize_sync(nc)
    return nc, dbg_outs


# ---------------------------------------------------------------- host side
def _bf(a):
    return np.asarray(a, dtype=np.float32).astype(ml_dtypes.bfloat16)


def _pcols(vec, ncols):
    """[ncols*128] -> [128, ncols], col j = vec[j*128:(j+1)*128]."""
    return np.ascontiguousarray(
        np.asarray(vec, dtype=np.float32).reshape(ncols, 128).T)


def prepare_inputs(inputs):
    X = np.asarray(inputs["X"], dtype=np.float32)
    S = np.asarray(inputs["S_state"], dtype=np.float32)
    bias = np.asarray(inputs["bias"], dtype=np.float32)
    tok2grid = np.asarray(inputs["tok2grid"]).astype(np.int64)
    Wb = np.asarray(inputs["Wb"], dtype=np.float32)
    Wq = np.asarray(inputs["Wq"], dtype=np.float32)
    Wk = np.asarray(inputs["Wk"], dtype=np.float32)
    Wv = np.asarray(inputs["Wv"], dtype=np.float32)
    Wo_ = np.asarray(inputs["Wo"], dtype=np.float32)
    fc1_W = np.asarray(inputs["fc1_W"], dtype=np.float32)
    fc2_W = np.asarray(inputs["fc2_W"], dtype=np.float32)
    conv_W = np.asarray(inputs["conv_W"], dtype=np.float32)
    bq = np.asarray(inputs["bq"], dtype=np.float32)
    bv = np.asarray(inputs["bv"], dtype=np.float32)

    shared = {}
    # block-diagonal Wb over m-chunks of 8: [(d,mi), (h,mj)] per layer
    wblk = np.zeros([NL, 8, DB, H, 8], np.float32)
    for mi in range(8):
        wblk[:, mi, :, :, mi] = Wb
    shared["wbblk"] = _bf(np.ascontiguousarray(
        wblk.transpose(0, 2, 1, 3, 4).reshape(NL, 128, 64)
        .transpose(1, 0, 2)))
    shared["wqkv"] = _bf(np.concatenate(
        [Wq.transpose(0, 2, 1, 3).reshape(NL, DD, H * DA),
         Wk.transpose(0, 2, 1, 3).reshape(NL, DD, H * DA),
         Wv.transpose(0, 2, 1, 3).reshape(NL, DD, H * DM)], axis=2))
    shared["wo"] = _bf(Wo_)
    shared["fc1w"] = _bf(fc1_W)
    shared["fc2w"] = _bf(fc2_W)
    cw = conv_W.transpose(0, 2, 3, 4, 1).reshape(LC, DD, 9 * DD)
    shared["cww"] = _bf(cw.reshape(LC, 2, 128, 9 * DD))

    lng = np.stack([
        *[_pcols(inputs["ln1_g"][l], 2) for l in range(NL)],
        *[_pcols(inputs["ln2_g"][l], 2) for l in range(NL)],
        *[_pcols(inputs["cln_g"][c], 2) for c in range(LC)]])
    lnbv = np.stack([
        *[_pcols(inputs["ln1_b"][l], 2) for l in range(NL)],
        *[_pcols(inputs["ln2_b"][l], 2) for l in range(NL)],
        *[_pcols(inputs["cln_b"][c], 2) for c in range(LC)]])
    order = [0, 3, 1, 4, 2, 5, 6, 7, 8]
    shared["lngP"] = np.ascontiguousarray(lng[order].transpose(1, 0, 2))
    shared["lnbP"] = np.ascontiguousarray(lnbv[order].transpose(1, 0, 2))
    shared["boP"] = np.ascontiguousarray(np.stack(
        [_pcols(inputs["bo"][l], 2) for l in range(NL)], axis=1))
    shared["b1P"] = np.ascontiguousarray(np.stack(
        [_pcols(inputs["fc1_b"][l], 8) for l in range(NL)], axis=1))
    shared["b2P"] = np.ascontiguousarray(np.stack(
        [_pcols(inputs["fc2_b"][l], 2) for l in range(NL)], axis=1))
    shared["cbP"] = np.ascontiguousarray(np.stack(
        [_pcols(inputs["conv_b"][c], 2) for c in range(NL)], axis=1))
    shared["pP"] = _pcols(inputs["P"], 2)
    bqf = bq.reshape(NL, H * DA)
    shared["bqP"] = np.ascontiguousarray(np.stack(
        [_pcols(bqf[l], 4) for l in range(NL)], axis=1))
    bvf = bv.reshape(NL, H * DM)
    shared["bvP"] = _bf(np.ascontiguousarray(np.stack(
        [_pcols(bvf[l], 4) for l in range(NL)], axis=1)))
    shared["eyeD"] = _bf(np.eye(128, dtype=np.float32))
    obt = np.zeros([16, 16, 64], np.float32)
    for b in range(16):
        obt[b, b, :] = 1.0
    shared["obtD"] = _bf(obt.reshape(16, 1024))

    in_maps = []
    for c in range(NCORES):
        bs = slice(c * BL, (c + 1) * BL)
        m = dict(shared)
        rs2 = 1.0 / math.sqrt(2.0)
        m["xT"] = np.ascontiguousarray(
            X[bs].transpose(2, 0, 1).reshape(DD, R)) * rs2
        m["sT"] = np.ascontiguousarray(
            S[bs].transpose(2, 0, 1).reshape(DD, R)) * rs2
        # [mc, (d,mi), (b,l)]: biasT2[mc, d*8+mi, b*L+l] = bias[b,l,8mc+mi,d]
        bt2 = bias[bs].transpose(2, 3, 0, 1)          # [m, d, b, l]
        bt2 = bt2.reshape(MC, 8, DB, BL, L)           # [mc, mi, d, b, l]
        bt2 = bt2.transpose(0, 2, 1, 3, 4).reshape(MC, 128, BL * L)
        m["biasT2"] = _bf(np.ascontiguousarray(bt2))
        in_maps.append(m)
    return in_maps, tok2grid


_GRAPH_CACHE = {}


def kernel(**inputs):
    in_maps, tok2grid = prepare_inputs(inputs)
    key = tuple(int(v) for v in tok2grid)
    if key not in _GRAPH_CACHE:
        _GRAPH_CACHE[key] = build_graph(tok2grid)[0]
    nc = _GRAPH_CACHE[key]
    res = run_bass_kernel_spmd(nc, in_maps, core_ids=list(range(NCORES)))
    outs = []
    for c in range(NCORES):
        o = res.results[c]["out"].reshape(DD, BL, L).transpose(1, 2, 0)
        outs.append(o)
    return np.ascontiguousarray(np.concatenate(outs, axis=0),
                                dtype=np.float32)


# revision 39
# speedup vs baseline: 1.1936x; 1.1787x over previous
"""AlphaQubit-like decoder on 8 TRN2 NeuronCores (Bass/Tile).

Sharding: data-parallel over batch (16 per core), zero collectives.
Activations are feature-major [d (partitions), (b, l) (free)] so every
matmul contracts on partitions with weights stationary and every bias is
per-partition. bf16 matmul operands; f32 residual stream + PSUM.

Structure (2.21ms -> 1.46ms vs the per-(b,h) baseline):
  - bq folded into Q at QKV PSUM evacuation (per-partition bias), bk
    dropped (softmax-invariant), bv folded via bo_eff = bo + Wo^T bv;
    the (X+S)/sqrt2 prescale is applied host-side.
  - Pairwise-bias projection Bp computed with a block-diagonal lhs over
    m-chunks of 8 so the output lands m-major ([*(h,mj), (b,l)]), then
    stored to DRAM as [NH, m, (b,l)] with fat contiguous rows; attention
    loads one [120, 1920] head tile per (l,h) with 3.8KB descriptors
    (the v1 thin-row transpose loads burned ~700us of DMA-queue time).
  - Attention batched per (l,h) in a 3-stage head pipeline: S^T in
    [120,480] PSUM quarters (+Bp via identity matmul), one exp per
    quarter; row-sums computed TRANSPOSED ([120(l),16(b)] via E-as-lhs
    N=1 matmuls) so the DVE reciprocal (6.3ns/free-elem!) runs on 16
    free elems instead of 46080; orientation restored by an
    identity-rhs transpose matmul + delta-broadcast matmuls; one DVE
    multiply per quarter normalizes A@V.
  - LayerNorm rs = exp(-0.5*ln(var+eps)) on the Activation engine (ln
    and exp share one act table with attention's exp; no DVE
    reciprocal); squares on DVE in bf16; mean/rs broadcast by K=1
    matmuls; chunk-staggered emission keeps the scalar rsqrt tail off
    the PE queue's head.
  - FFN gate: (fc1g + b) * gelu(fc1a + b) via one scalar_tensor_tensor.
  - Conv LN writer uses 3D-AP activations; conv matmuls in 3-batch
    N=432 chunks; scatter/gather and pad-zeroing split across engines
    and hoisted off the conv-entry critical path.
  - Bulk DMAs rotate over gpsimd/scalar queues; sync is reserved for
    bias-tile and Bp head loads so they never queue behind stores.
"""

import contextlib
import itertools
import math

import numpy as np
import ml_dtypes

import concourse.bass as bass
import concourse.mybir as mybir
import concourse.tile as tile
from concourse.bass_utils import run_bass_kernel_spmd

F32 = mybir.dt.float32
BF16 = mybir.dt.bfloat16
AX = mybir.AluOpType
AF = mybir.ActivationFunctionType

B, L, DD = 128, 120, 256
DA, DM, H, DB = 64, 64, 8, 16
NL, LC = 3, 3
DILS = (1, 2, 4)
G = 12
NCORES = 8
BL = B // NCORES            # 16 batches per core
R = BL * L                  # 1920 seq cols per core
CELLS = G * G               # 144
RG = BL * CELLS             # 2304 grid cols per core
NH = NL * H                 # 24 (layer,head) rows
SP = 20                     # padded grid side (pad=4 fits dilation<=4)
PADC = SP * SP
CHUNK = 480                 # seq chunk (4 batches)
NCH = R // CHUNK            # 4 seq chunks
GCH = 288                   # grid chunk (2 batches)
MC = 15                     # m-chunks of 8 in the Bp projection
EPS = 1e-5
SCALE = 1.0 / math.sqrt(DA)


def _legalize_sync(nc):
    """This walrus build caps per-instruction semaphore waits (drain: 0,
    matmul/ldweights: 1, others: 2); move excess waits onto NoOps."""
    caps = {"InstDrain": 0}
    for f in nc.m.functions:
        for bb in f.blocks:
            new_insts = []
            for inst in bb.instructions:
                si = getattr(inst, "sync_info", None)
                cap = caps.get(type(inst).__name__, 1)
                if si is not None and si.on_wait and len(si.on_wait) > cap:
                    waits = list(si.on_wait)
                    keep = waits[len(waits) - cap:] if cap else []
                    for i, w in enumerate(waits[: len(waits) - cap]):
                        new_insts.append(mybir.InstNoOp(
                            name=inst.name + f"-ws{i}", engine=inst.engine,
                            ins=[], outs=[],
                            sync_info=mybir.SyncInfo(on_wait=[w],
                                                     on_update=[])))
                    si.on_wait = keep
                new_insts.append(inst)
            bb.instructions = new_insts


def build_graph(tok2grid, debug_taps=()):
    dbg = set(debug_taps)
    nc = bass.Bass()

    xT = nc.dram_tensor("xT", [DD, R], F32, kind="ExternalInput")
    sT = nc.dram_tensor("sT", [DD, R], F32, kind="ExternalInput")
    biasT2 = nc.dram_tensor("biasT2", [MC, 128, R], BF16,
                            kind="ExternalInput")
    wbblk = nc.dram_tensor("wbblk", [128, NL, 64], BF16,
                           kind="ExternalInput")
    wqkv = nc.dram_tensor("wqkv", [NL, DD, 1536], BF16, kind="ExternalInput")
    wo = nc.dram_tensor("wo", [NL, H * DM, DD], BF16, kind="ExternalInput")
    fc1w = nc.dram_tensor("fc1w", [NL, DD, 1024], BF16, kind="ExternalInput")
    fc2w = nc.dram_tensor("fc2w", [NL, 512, DD], BF16, kind="ExternalInput")
    cww = nc.dram_tensor("cww", [LC, 2, 128, 9 * DD], BF16,
                         kind="ExternalInput")
    lngP = nc.dram_tensor("lngP", [128, 9, 2], F32, kind="ExternalInput")
    lnbP = nc.dram_tensor("lnbP", [128, 9, 2], F32, kind="ExternalInput")
    boP = nc.dram_tensor("boP", [128, NL, 2], F32, kind="ExternalInput")
    b1P = nc.dram_tensor("b1P", [128, NL, 8], F32, kind="ExternalInput")
    b2P = nc.dram_tensor("b2P", [128, NL, 2], F32, kind="ExternalInput")
    cbP = nc.dram_tensor("cbP", [128, LC, 2], F32, kind="ExternalInput")
    pP = nc.dram_tensor("pP", [128, 2], F32, kind="ExternalInput")
    bqP = nc.dram_tensor("bqP", [128, NL, 4], F32, kind="ExternalInput")
    bvP = nc.dram_tensor("bvP", [128, NL, 4], BF16, kind="ExternalInput")
    eyeD = nc.dram_tensor("eyeD", [128, 128], BF16, kind="ExternalInput")
    obtD = nc.dram_tensor("obtD", [16, 1024], BF16, kind="ExternalInput")
    outD = nc.dram_tensor("out", [DD, R], F32, kind="ExternalOutput")

    dbg_outs = {}

    def dbg_tensor(name, shape):
        dbg_outs[name] = nc.dram_tensor(name, shape, F32,
                                        kind="ExternalOutput")
        return dbg_outs[name]

    # token<->grid runs (host-known permutation baked into APs)
    t2g = [int(v) for v in tok2grid]
    assert len(set(t2g)) == L and all(0 <= v < CELLS for v in t2g)
    runs = []
    s = 0
    for i in range(1, L + 1):
        if i == L or t2g[i] != t2g[i - 1] + 1:
            runs.append((s, t2g[s], i - s))
            s = i
    unmapped = sorted(set(range(CELLS)) - set(t2g))
    uruns = []
    if unmapped:
        s = 0
        for i in range(1, len(unmapped) + 1):
            if i == len(unmapped) or unmapped[i] != unmapped[i - 1] + 1:
                uruns.append((unmapped[s], i - s))
                s = i

    uid = [0]

    def pool(lctx, name, bufs, space="SBUF"):
        uid[0] += 1
        return lctx.enter_context(
            tc.tile_pool(name=f"{name}_{uid[0]}", bufs=bufs, space=space))

    # rotate bulk DMAs across engine queues
    qrr = itertools.cycle(["gpsimd", "scalar"])

    def dma(out, in_):
        getattr(nc, next(qrr)).dma_start(out=out, in_=in_)

    with tile.TileContext(nc) as tc, contextlib.ExitStack() as ctx:
        wp = ctx.enter_context(tc.tile_pool(name="weights", bufs=1))
        xp = ctx.enter_context(tc.tile_pool(name="xres", bufs=2))
        dram = ctx.enter_context(tc.tile_pool(name="dram", bufs=1,
                                              space="DRAM"))

        # ---------------- weights/constants to SBUF
        w_qkv = [[wp.tile([128, 1536], BF16, tag=f"wqkv{l}{k}",
                          name=f"wqkv{l}{k}") for k in range(2)]
                 for l in range(NL)]
        w_o = [[wp.tile([128, DD], BF16, tag=f"wo{l}{k}", name=f"wo{l}{k}")
                for k in range(4)] for l in range(NL)]
        w_f1 = [[wp.tile([128, 1024], BF16, tag=f"f1{l}{k}", name=f"f1{l}{k}")
                 for k in range(2)] for l in range(NL)]
        w_f2 = [[wp.tile([128, DD], BF16, tag=f"f2{l}{k}", name=f"f2{l}{k}")
                 for k in range(4)] for l in range(NL)]
        w_cv = [[wp.tile([128, 9 * DD], BF16, tag=f"cv{c}{k}",
                         name=f"cv{c}{k}") for k in range(2)]
                for c in range(LC)]
        for k in range(2):
            dma(w_qkv[0][k][:], wqkv[0, 128 * k:128 * (k + 1), :])
        for l in range(NL):
            for k in range(4):
                dma(w_o[l][k][:], wo[l, 128 * k:128 * (k + 1), :])

        def load_late_weights():
            for l in range(NL):
                for k in range(2):
                    if l > 0:
                        dma(w_qkv[l][k][:],
                            wqkv[l, 128 * k:128 * (k + 1), :])
                    dma(w_f1[l][k][:], fc1w[l, 128 * k:128 * (k + 1), :])
                for k in range(4):
                    dma(w_f2[l][k][:], fc2w[l, 128 * k:128 * (k + 1), :])
            for c in range(LC):
                for k in range(2):
                    dma(w_cv[c][k][:], cww[c, k, :, :])

        lng = wp.tile([128, 9, 2], F32)
        dma(lng[:], lngP[:])
        lnb = wp.tile([128, 9, 2], F32)
        dma(lnb[:], lnbP[:])
        bo_s = wp.tile([128, NL, 2], F32)
        dma(bo_s[:], boP[:])
        b1_s = wp.tile([128, NL, 8], F32)
        dma(b1_s[:], b1P[:])
        b2_s = wp.tile([128, NL, 2], F32)
        dma(b2_s[:], b2P[:])
        cb_s = wp.tile([128, LC, 2], F32)
        dma(cb_s[:], cbP[:])
        p_s = wp.tile([128, 2], F32)
        dma(p_s[:], pP[:])
        bq_s = wp.tile([128, NL, 4], F32)
        dma(bq_s[:], bqP[:])
        bv_s = wp.tile([128, NL, 4], BF16)
        dma(bv_s[:], bvP[:])
        eye_s = wp.tile([128, 128], BF16)
        dma(eye_s[:], eyeD[:])
        obt_s = wp.tile([16, 1024], BF16)
        dma(obt_s[:], obtD[:])
        wbb_s = wp.tile([128, NL, 64], BF16)
        dma(wbb_s[:], wbblk[:])

        ones_rowb = wp.tile([1, 128], BF16)
        nc.vector.memset(ones_rowb[:], 1.0)
        invd_col = wp.tile([128, 1], BF16)
        nc.vector.memset(invd_col[:], 1.0 / DD)
        onec = wp.tile([128, 1], BF16)
        nc.vector.memset(onec[:], 1.0)
        eps_col = wp.tile([128, 1], F32)
        nc.vector.memset(eps_col[:], EPS)
        zero_col = wp.tile([128, 1], F32)
        nc.vector.memset(zero_col[:], 0.0)

        # fold bv into bo: bo_eff = bo + Wo^T bv
        bo_eff = wp.tile([128, NL, 2], F32)
        with tc.tile_pool(name="prep_ps", bufs=2, space="PSUM") as pps:
            for l in range(NL):
                d_ps = pps.tile([128, 2], F32, tag="dps", name="dps")
                for ot in range(2):
                    for k in range(4):
                        nc.tensor.matmul(
                            d_ps[:, ot:ot + 1],
                            w_o[l][k][:, 128 * ot:128 * (ot + 1)],
                            bv_s[:, l, k:k + 1],
                            start=(k == 0), stop=(k == 3))
                nc.vector.tensor_add(out=bo_eff[:, l], in0=d_ps[:],
                                     in1=bo_s[:, l])

        # ---------------- residual init: x = X + S (host pre-scaled 1/sqrt2)
        X = [[xp.tile([128, R], F32, tag=f"X{t}", name=f"X{t}")
              for t in range(2)]]
        with contextlib.ExitStack() as lctx:
            ipool = pool(lctx, "init", 2)
            for t in range(2):
                xi = ipool.tile([128, R], F32, tag="xi", name="xi")
                si = ipool.tile([128, R], F32, tag="si", name="si")
                nc.sync.dma_start(out=xi[:], in_=xT[128 * t:128 * (t + 1), :])
                nc.sync.dma_start(out=si[:], in_=sT[128 * t:128 * (t + 1), :])
                nc.vector.tensor_add(out=X[0][t][:], in0=xi[:], in1=si[:])
        if "x0" in dbg:
            d = dbg_tensor("d_x0", [DD, R])
            for t in range(2):
                nc.sync.dma_start(out=d[128 * t:128 * (t + 1), :],
                                  in_=X[0][t][:])

        # ---------------- layernorm (feature-major, chunk-staggered)
        def layernorm(lctx, li, src, n_chunks, ccols, writer):
            """src: 2 f32 tiles [128, n_chunks*ccols]. writer(t, c, xc, rsb)
            must emit (xc * g) * rs + b into its target; xc f32 SBUF
            [128, ccols], rsb f32 PSUM [128, ccols] broadcast of rs.
            Stats matmuls of chunk c+1 are emitted before the broadcast of
            chunk c so the PE queue never stalls on the scalar rsqrt tail."""
            sps = pool(lctx, "ln_st", 2, "PSUM")
            bps = pool(lctx, "ln_bc", 2, "PSUM")
            sp = pool(lctx, "ln_sb", 2)

            def stats(c):
                cs = slice(c * ccols, (c + 1) * ccols)
                xb = [sp.tile([128, 2, ccols], BF16, tag=f"xb{t}",
                              name=f"xb{t}") for t in range(2)]
                for t in range(2):
                    nc.gpsimd.dma_start(out=xb[t][:, 0], in_=src[t][:, cs])
                for t in range(2):   # squares on DVE (bf16 fast mode)
                    nc.vector.tensor_mul(out=xb[t][:, 1], in0=xb[t][:, 0],
                                         in1=xb[t][:, 0])
                st = sps.tile([1, 2, 512], F32, tag="st", name="st")
                for t in range(2):
                    nc.tensor.matmul(st[:, 0, :ccols], invd_col[:],
                                     xb[t][:, 0], start=(t == 0),
                                     stop=(t == 1))
                for t in range(2):
                    nc.tensor.matmul(st[:, 1, :ccols], invd_col[:],
                                     xb[t][:, 1], start=(t == 0),
                                     stop=(t == 1))
                return st

            def tail(c, st):
                cs = slice(c * ccols, (c + 1) * ccols)
                mrs = sp.tile([1, 2, ccols], BF16, tag="mrs", name="mrs")
                nc.scalar.copy(mrs[:, 0], st[:, 0, :ccols])
                sqm = sp.tile([1, ccols], F32, tag="sqm", name="sqm")
                nc.scalar.square(sqm[:], mrs[:, 0])
                lnv = sp.tile([1, ccols], F32, tag="lnv", name="lnv")
                nc.vector.tensor_sub(out=lnv[:], in0=st[:, 1, :ccols],
                                     in1=sqm[:])
                # rs = (var+eps)^-1/2 = exp(-0.5*ln(var+eps))
                nc.scalar.activation(lnv[:], lnv[:], AF.Ln,
                                     bias=eps_col[0:1, :], scale=1.0)
                nc.scalar.activation(mrs[:, 1], lnv[:], AF.Exp,
                                     bias=zero_col[0:1, :], scale=-0.5)
                mb = bps.tile([128, ccols], F32, tag="mb", name="mb")
                nc.tensor.matmul(mb[:], ones_rowb[:], mrs[:, 0],
                                 start=True, stop=True)
                rsb = bps.tile([128, ccols], F32, tag="rsb", name="rsb")
                nc.tensor.matmul(rsb[:], ones_rowb[:], mrs[:, 1],
                                 start=True, stop=True)
                for t in range(2):
                    xc = sp.tile([128, ccols], BF16, tag=f"xc{t}",
                                 name=f"xc{t}")
                    nc.vector.tensor_sub(out=xc[:], in0=src[t][:, cs],
                                         in1=mb[:])
                    writer(t, c, xc, rsb)

            prev = None
            for c in range(n_chunks):
                st = stats(c)
                if prev is not None:
                    tail(*prev)
                prev = (c, st)
            tail(*prev)

        # ---------------- Bp projection: m-major blocks -> DRAM
        BpTd = dram.tile([NH, 120, R], BF16)

        def emit_bp_projection():
            with contextlib.ExitStack() as lctx:
                btp = pool(lctx, "bp_in", 3)
                bpp = pool(lctx, "bp_ps", 8, "PSUM")
                bst = pool(lctx, "bp_st", 3)
                for mc in range(MC):
                    bt = btp.tile([128, R], BF16, tag="bt", name="bt")
                    nc.sync.dma_start(out=bt[:], in_=biasT2[mc])
                    for l in range(NL):
                        stage = bst.tile([64, R], BF16, tag="bstg",
                                         name="bstg")
                        for q in range(NCH):
                            qs = slice(q * CHUNK, (q + 1) * CHUNK)
                            ps = bpp.tile([64, CHUNK], F32, tag="bpp",
                                          name="bpp")
                            nc.tensor.matmul(ps[:], wbb_s[:, l, :],
                                             bt[:, qs],
                                             start=True, stop=True)
                            if (mc + l + q) % 2 == 0:
                                nc.scalar.copy(stage[:, qs], ps[:])
                            else:
                                nc.vector.tensor_copy(out=stage[:, qs],
                                                      in_=ps[:])
                        for h in range(H):
                            dma(BpTd[8 * l + h, 8 * mc:8 * (mc + 1), :],
                                stage[8 * h:8 * (h + 1), :])

        load_late_weights()

        if dbg:   # debug taps need the projection data eagerly
            emit_bp_projection()
        if "bp" in dbg:
            d_bp = nc.dram_tensor("d_bp", [NH, 120, R], BF16,
                                  kind="ExternalOutput")
            dbg_outs["d_bp"] = d_bp
            nc.sync.dma_start(out=d_bp[:], in_=BpTd[:])

        # ---------------- layers
        for l in range(NL):
            Xc = X[l]
            with contextlib.ExitStack() as lp:
                op_ = pool(lp, f"op{l}", 1)    # O^T: lives till Wo
                O = [op_.tile([128, R], BF16, tag=f"o{ot}", name=f"o{ot}")
                     for ot in range(4)]
                qa = contextlib.ExitStack()
                lp.enter_context(qa)
                qv = pool(qa, f"qv{l}", 1)   # qk/V live till attn end
                qk = [qv.tile([128, R], BF16, tag=f"qk{ot}",
                              name=f"qk{ot}") for ot in range(8)]
                V = [qv.tile([120, 512], BF16, tag=f"v{b}",
                             name=f"v{b}") for b in range(BL)]
                # ---- LN1 + QKV (Xn scoped)
                with contextlib.ExitStack() as ph:
                    xnp = pool(ph, f"xn{l}", 1)
                    Xn = [xnp.tile([128, R], BF16, tag=f"xn{t}",
                                   name=f"xn{t}") for t in range(2)]

                    def w_ln1(t, c, xc, rsb, l=l, Xn=Xn):
                        tgt = Xn[t][:, c * CHUNK:(c + 1) * CHUNK]
                        nc.vector.tensor_mul(out=xc[:], in0=xc[:],
                                             in1=rsb[:])
                        nc.vector.tensor_scalar(
                            out=tgt, in0=xc[:],
                            scalar1=lng[:, 2 * l, t:t + 1],
                            scalar2=lnb[:, 2 * l, t:t + 1],
                            op0=AX.mult, op1=AX.add)

                    with contextlib.ExitStack() as lnx:
                        layernorm(lnx, 2 * l, Xc, NCH, CHUNK, w_ln1)
                    if l == 0 and "ln1" in dbg:
                        d = dbg_tensor("d_ln1", [DD, R])
                        for t in range(2):
                            nc.gpsimd.dma_start(
                                out=d[128 * t:128 * (t + 1), :],
                                in_=Xn[t][:])

                    qps = pool(ph, f"qkv_ps{l}", 2, "PSUM")
                    for c in range(NCH):
                        cs = slice(c * CHUNK, (c + 1) * CHUNK)
                        for ot in range(8):
                            ps = qps.tile([128, CHUNK], F32, tag="qkp",
                                          name="qkp", bufs=5)
                            for k in range(2):
                                nc.tensor.matmul(
                                    ps[:],
                                    w_qkv[l][k][:, 128 * ot:128 * (ot + 1)],
                                    Xn[k][:, cs], start=(k == 0),
                                    stop=(k == 1))
                            if ot < 4:   # Q tiles: fold bq in the evac
                                nc.scalar.activation(
                                    qk[ot][:, cs], ps[:], AF.Identity,
                                    bias=bq_s[:, l, ot:ot + 1], scale=1.0)
                            else:
                                nc.vector.tensor_copy(out=qk[ot][:, cs],
                                                      in_=ps[:])
                    for b in range(BL):
                        bs = slice(b * L, (b + 1) * L)
                        ps = qps.tile([120, 512], F32, tag="vp", name="vp")
                        for k in range(2):
                            nc.tensor.matmul(
                                ps[:], Xn[k][:, bs],
                                w_qkv[l][k][:, 1024:1536],
                                start=(k == 0), stop=(k == 1))
                        if b % 2 == 0:
                            nc.scalar.copy(V[b][:], ps[:])
                        else:
                            nc.vector.tensor_copy(out=V[b][:], in_=ps[:])

                if l == 0 and "qkv" in dbg:
                    dq = dbg_tensor("d_q0", [512, R])
                    dk = dbg_tensor("d_k0", [512, R])
                    for ot in range(4):
                        nc.gpsimd.dma_start(
                            out=dq[128 * ot:128 * (ot + 1), :],
                            in_=qk[ot][:])
                        nc.gpsimd.dma_start(
                            out=dk[128 * ot:128 * (ot + 1), :],
                            in_=qk[4 + ot][:])
                    dv = dbg_tensor("d_v0", [BL * 120, 512])
                    for b in range(BL):
                        nc.gpsimd.dma_start(
                            out=dv[120 * b:120 * (b + 1), :], in_=V[b][:])

                if l == 0 and not dbg:
                    emit_bp_projection()

                # ---- attention (batched per head, 3-stage head pipeline)
                with contextlib.ExitStack() as ph:
                    bpl = pool(ph, f"bpl{l}", 3)
                    epl = pool(ph, f"ep{l}", 3)
                    smp = pool(ph, f"sm{l}", 2)
                    spsp = pool(ph, f"attS{l}", 2, "PSUM")
                    rtp = pool(ph, f"attRT{l}", 1, "PSUM")
                    rbtp = pool(ph, f"attRB{l}", 1, "PSUM")
                    rbp = pool(ph, f"attRb{l}", 2, "PSUM")
                    ops_ = pool(ph, f"attO{l}", 2, "PSUM")
                    st_e = {}
                    st_rbt = {}

                    def stage_a(h, l=l):
                        qt, qb = h // 2, (h % 2) * 64
                        Bph = bpl.tile([120, R], BF16, tag="bph",
                                       name="bph")
                        nc.sync.dma_start(out=Bph[:], in_=BpTd[8 * l + h])
                        # exp(s+Bp) = exp(s) * exp(Bp): precompute the bias
                        # factor once per head, multiply on DVE in bf16
                        EB = bpl.tile([120, R], BF16, tag="eb", name="eb")
                        nc.scalar.activation(EB[:], Bph[:], AF.Exp,
                                             bias=zero_col[:120, :],
                                             scale=SCALE)
                        E = epl.tile([120, R], BF16, tag="E", name="E")
                        for q in range(NCH):
                            qs = slice(q * CHUNK, (q + 1) * CHUNK)
                            sps_t = spsp.tile([120, CHUNK], F32, tag="sps",
                                              name="sps")
                            for bi in range(4):
                                b = q * 4 + bi
                                bs = slice(b * L, (b + 1) * L)
                                nc.tensor.matmul(
                                    sps_t[:, bi * L:(bi + 1) * L],
                                    qk[4 + qt][qb:qb + 64, bs],
                                    qk[qt][qb:qb + 64, bs],
                                    start=True, stop=True)
                            nc.scalar.activation(
                                E[:, qs], sps_t[:], AF.Exp,
                                bias=zero_col[:120, :], scale=SCALE)
                            nc.vector.tensor_mul(out=E[:, qs], in0=E[:, qs],
                                                 in1=EB[:, qs])
                        st_e[h] = E

                    def stage_b(h):
                        E = st_e[h]
                        rT = rtp.tile([120, 16], F32, tag="rT", name="rT")
                        for b in range(BL):
                            nc.tensor.matmul(
                                rT[:, b:b + 1], E[:, b * L:(b + 1) * L],
                                onec[:120, :], start=True, stop=True)
                        rTi = smp.tile([120, 16], BF16, tag="rTi",
                                       name="rTi")
                        with nc.allow_low_precision(
                                reason="softmax 1/Z in bf16 feeds bf16 "
                                       "matmul broadcast"):
                            nc.vector.reciprocal(out=rTi[:], in_=rT[:])
                        rbt_ps = rbtp.tile([16, 120], F32, tag="rbt",
                                           name="rbt")
                        nc.tensor.matmul(rbt_ps[:], rTi[:],
                                         eye_s[:120, :120],
                                         start=True, stop=True)
                        rbt = smp.tile([16, 120], BF16, tag="rbs",
                                       name="rbs")
                        nc.scalar.copy(rbt[:], rbt_ps[:])
                        st_rbt[h] = rbt

                    def stage_c(h):
                        qt, qb = h // 2, (h % 2) * 64
                        E, rbt = st_e.pop(h), st_rbt.pop(h)
                        for q in range(NCH):
                            qs = slice(q * CHUNK, (q + 1) * CHUNK)
                            rb_t = rbp.tile([64, CHUNK], F32, tag="rbq",
                                            name="rbq")
                            o_t = ops_.tile([64, CHUNK], F32, tag="opq",
                                            name="opq")
                            for bi in range(4):
                                b = q * 4 + bi
                                nc.tensor.matmul(
                                    rb_t[:, bi * L:(bi + 1) * L],
                                    obt_s[:, 64 * b:64 * (b + 1)], rbt[:],
                                    start=True, stop=True)
                                nc.tensor.matmul(
                                    o_t[:, bi * L:(bi + 1) * L],
                                    V[b][:, 64 * h:64 * (h + 1)],
                                    E[:, b * L:(b + 1) * L],
                                    start=True, stop=True)
                            rb_sb = smp.tile([64, CHUNK], BF16, tag="rbb",
                                             name="rbb")
                            if (h + q) % 2 == 0:
                                nc.scalar.copy(rb_sb[:], rb_t[:])
                            else:
                                nc.vector.tensor_copy(out=rb_sb[:],
                                                      in_=rb_t[:])
                            nc.vector.tensor_mul(
                                out=O[qt][qb:qb + 64, qs], in0=o_t[:],
                                in1=rb_sb[:])

                    for h in range(H):
                        stage_a(h)
                        if h >= 1:
                            stage_b(h - 1)
                        if h >= 2:
                            stage_c(h - 2)
                    stage_b(H - 1)
                    stage_c(H - 2)
                    stage_c(H - 1)
                qa.close()   # free qk/V
                if l == 0 and "att" in dbg:
                    do = dbg_tensor("d_att0", [512, R])
                    for ot in range(4):
                        nc.gpsimd.dma_start(
                            out=do[128 * ot:128 * (ot + 1), :], in_=O[ot][:])

                # ---- Wo + residual
                Xa = [xp.tile([128, R], F32, tag=f"X{t}", name=f"X{t}")
                      for t in range(2)]
                with contextlib.ExitStack() as ph:
                    wps = pool(ph, f"wo_ps{l}", 6, "PSUM")
                    for c in range(NCH):
                        cs = slice(c * CHUNK, (c + 1) * CHUNK)
                        for ot in range(2):
                            ps = wps.tile([128, CHUNK], F32, tag="wop",
                                          name="wop")
                            for k in range(4):
                                nc.tensor.matmul(
                                    ps[:],
                                    w_o[l][k][:, 128 * ot:128 * (ot + 1)],
                                    O[k][:, cs], start=(k == 0),
                                    stop=(k == 3))
                            nc.vector.scalar_tensor_tensor(
                                out=Xa[ot][:, cs], in0=ps[:],
                                scalar=bo_eff[:, l, ot:ot + 1],
                                in1=Xc[ot][:, cs], op0=AX.add, op1=AX.add)
                if l == 0 and "wo" in dbg:
                    d = dbg_tensor("d_wo0", [DD, R])
                    for t in range(2):
                        nc.sync.dma_start(out=d[128 * t:128 * (t + 1), :],
                                          in_=Xa[t][:])

                # ---- LN2 + gated FFN (pad grid zeroed here: fills queue
                # slack long before the conv block reads it)
                gp = pool(lp, f"grid{l}", 1)
                pad = [gp.tile([128, BL * PADC], BF16, tag=f"pad{t}",
                               name=f"pad{t}") for t in range(2)]
                nc.vector.memset(pad[0][:], 0.0)
                nc.gpsimd.memset(pad[1][:], 0.0)
                Xf = [xp.tile([128, R], F32, tag=f"X{t}", name=f"X{t}")
                      for t in range(2)]
                with contextlib.ExitStack() as ph:
                    xnp = pool(ph, f"xn2{l}", 1)
                    Xn2 = [xnp.tile([128, R], BF16, tag=f"xn2{t}",
                                    name=f"xn2{t}") for t in range(2)]

                    def w_ln2(t, c, xc, rsb, l=l, Xn2=Xn2):
                        tgt = Xn2[t][:, c * CHUNK:(c + 1) * CHUNK]
                        nc.vector.tensor_mul(out=xc[:], in0=xc[:],
                                             in1=rsb[:])
                        nc.scalar.activation(
                            tgt, xc[:], AF.Identity,
                            bias=lnb[:, 2 * l + 1, t:t + 1],
                            scale=lng[:, 2 * l + 1, t:t + 1])

                    with contextlib.ExitStack() as lnx:
                        layernorm(lnx, 2 * l + 1, Xa, NCH, CHUNK, w_ln2)

                    hp_ = pool(ph, f"hh{l}", 1)
                    Hh = [hp_.tile([128, R], BF16, tag=f"h{ot}",
                                   name=f"h{ot}") for ot in range(4)]
                    fps = pool(ph, f"f1ps{l}", 3, "PSUM")
                    fsb = pool(ph, f"f1sb{l}", 3)
                    for c in range(NCH):
                        cs = slice(c * CHUNK, (c + 1) * CHUNK)
                        for ot in range(4):
                            ps = fps.tile([128, CHUNK], F32, tag="f1a",
                                          name="f1a")
                            for k in range(2):
                                nc.tensor.matmul(
                                    ps[:],
                                    w_f1[l][k][:, 128 * ot:128 * (ot + 1)],
                                    Xn2[k][:, cs], start=(k == 0),
                                    stop=(k == 1))
                            ga = fsb.tile([128, CHUNK], BF16, tag="ga",
                                          name="ga")
                            nc.scalar.activation(
                                ga[:], ps[:], AF.Gelu,
                                bias=b1_s[:, l, ot:ot + 1], scale=1.0)
                            ps2 = fps.tile([128, CHUNK], F32, tag="f1g",
                                           name="f1g")
                            for k in range(2):
                                nc.tensor.matmul(
                                    ps2[:],
                                    w_f1[l][k][:, 512 + 128 * ot:
                                               512 + 128 * (ot + 1)],
                                    Xn2[k][:, cs], start=(k == 0),
                                    stop=(k == 1))
                            nc.vector.scalar_tensor_tensor(
                                out=Hh[ot][:, cs], in0=ps2[:],
                                scalar=b1_s[:, l, 4 + ot:5 + ot],
                                in1=ga[:], op0=AX.add, op1=AX.mult)
                    f2ps = pool(ph, f"f2ps{l}", 2, "PSUM")
                    for c in range(NCH):
                        cs = slice(c * CHUNK, (c + 1) * CHUNK)
                        for ot in range(2):
                            ps = f2ps.tile([128, CHUNK], F32, tag="f2p",
                                           name="f2p")
                            for k in range(4):
                                nc.tensor.matmul(
                                    ps[:],
                                    w_f2[l][k][:, 128 * ot:128 * (ot + 1)],
                                    Hh[k][:, cs], start=(k == 0),
                                    stop=(k == 3))
                            nc.vector.scalar_tensor_tensor(
                                out=Xf[ot][:, cs], in0=ps[:],
                                scalar=b2_s[:, l, ot:ot + 1],
                                in1=Xa[ot][:, cs], op0=AX.add, op1=AX.add)
                if l == 0 and "ffn" in dbg:
                    d = dbg_tensor("d_ffn0", [DD, R])
                    for t in range(2):
                        nc.sync.dma_start(out=d[128 * t:128 * (t + 1), :],
                                          in_=Xf[t][:])

                # ---- conv block (shared params)
                with contextlib.ExitStack() as ph:
                    Gt = [gp.tile([128, RG], BF16, tag=f"g{t}",
                                  name=f"g{t}") for t in range(2)]
                    for t in range(2):
                        xv = Xf[t][:].rearrange("p (b l) -> p b l", b=BL)
                        gv = Gt[t][:].rearrange("p (b c) -> p b c", b=BL)
                        for (ts_, cs_, ln_) in runs:
                            if t == 0:
                                nc.scalar.copy(gv[:, :, cs_:cs_ + ln_],
                                               xv[:, :, ts_:ts_ + ln_])
                            else:
                                nc.vector.tensor_copy(
                                    out=gv[:, :, cs_:cs_ + ln_],
                                    in_=xv[:, :, ts_:ts_ + ln_])
                        for (cs_, ln_) in uruns:
                            nc.scalar.activation(
                                gv[:, :, cs_:cs_ + ln_],
                                xv[:, :, 0:ln_], AF.Identity,
                                bias=p_s[:, t:t + 1], scale=0.0)

                    for cv in range(LC):
                        dil = DILS[cv]

                        def w_cln(t, c, xc, rsb, cv=cv):
                            b0 = c * 2
                            pv = pad[t][:].rearrange(
                                "p (b y x) -> p b y x", y=SP, x=SP)
                            nc.vector.tensor_mul(out=xc[:], in0=xc[:],
                                                 in1=rsb[:])
                            xcv = xc[:].rearrange(
                                "p (b y x) -> p b y x", y=G, x=G)
                            nc.vector.tensor_scalar(
                                out=pv[:, b0:b0 + 2, 4:16, 4:16],
                                in0=xcv[:],
                                scalar1=lng[:, 6 + cv, t:t + 1],
                                scalar2=lnb[:, 6 + cv, t:t + 1],
                                op0=AX.mult, op1=AX.add)

                        with contextlib.ExitStack() as lnx:
                            layernorm(lnx, 6 + cv, Gt, RG // GCH, GCH,
                                      w_cln)
                        with contextlib.ExitStack() as cvx:
                            cps = pool(cvx, f"cvps{l}{cv}", 6, "PSUM")
                            csb = pool(cvx, f"cvsb{l}{cv}", 4)
                            chunks = [(0, 3), (3, 3), (6, 3),
                                      (9, 3), (12, 3), (15, 1)]
                            for grp in range(2):
                                gch = chunks[3 * grp:3 * (grp + 1)]
                                pss = {}
                                for ot in range(2):
                                    for ci in range(3):
                                        pss[(ot, ci)] = cps.tile(
                                            [128, 432], F32, tag="cvp",
                                            name="cvp")
                                for ot in range(2):
                                    for tap in range(9):
                                        ky, kx = tap // 3, tap % 3
                                        y0 = 4 + (ky - 1) * dil
                                        x0 = 4 + (kx - 1) * dil
                                        for k in range(2):
                                            lhs = w_cv[cv][k][
                                                :, tap * DD + 128 * ot:
                                                tap * DD + 128 * (ot + 1)]
                                            for ci, (b0, nb) in \
                                                    enumerate(gch):
                                                pv = pad[k][:].rearrange(
                                                    "p (b y x) -> p b y x",
                                                    y=SP, x=SP)
                                                rhs = pv[:, b0:b0 + nb,
                                                         y0:y0 + G,
                                                         x0:x0 + G]
                                                nc.tensor.matmul(
                                                    pss[(ot, ci)]
                                                    [:, :nb * CELLS],
                                                    lhs, rhs,
                                                    start=(tap == 0 and
                                                           k == 0),
                                                    stop=(tap == 8 and
                                                          k == 1))
                                for ot in range(2):
                                    for ci, (b0, nb) in enumerate(gch):
                                        gs = slice(b0 * CELLS,
                                                   (b0 + nb) * CELLS)
                                        nn_ = nb * CELLS
                                        tt = csb.tile([128, 432], BF16,
                                                      tag="cvt", name="cvt")
                                        nc.scalar.activation(
                                            tt[:, :nn_],
                                            pss[(ot, ci)][:, :nn_],
                                            AF.Gelu,
                                            bias=cb_s[:, cv, ot:ot + 1],
                                            scale=1.0)
                                        nc.vector.tensor_add(
                                            out=Gt[ot][:, gs],
                                            in0=Gt[ot][:, gs],
                                            in1=tt[:, :nn_])

                    # gather tokens back
                    Xo = [xp.tile([128, R], F32, tag=f"X{t}", name=f"X{t}")
                          for t in range(2)]
                    for t in range(2):
                        xv = Xo[t][:].rearrange("p (b l) -> p b l", b=BL)
                        gv = Gt[t][:].rearrange("p (b c) -> p b c", b=BL)
                        for (ts_, cs_, ln_) in runs:
                            if t == 0:
                                nc.scalar.copy(xv[:, :, ts_:ts_ + ln_],
                                               gv[:, :, cs_:cs_ + ln_])
                            else:
                                nc.vector.tensor_copy(
                                    out=xv[:, :, ts_:ts_ + ln_],
                                    in_=gv[:, :, cs_:cs_ + ln_])
                    X.append(Xo)
                if l == 0 and "conv" in dbg:
                    d = dbg_tensor("d_conv0", [DD, R])
                    for t in range(2):
                        nc.sync.dma_start(out=d[128 * t:128 * (t + 1), :],
                                          in_=X[l + 1][t][:])

        for t in range(2):
            nc.sync.dma_start(out=outD[128 * t:128 * (t + 1), :],
                              in_=X[NL][t][:])

    _legalize_sync(nc)
    return nc, dbg_outs


# ---------------------------------------------------------------- host side
def _bf(a):
    return np.asarray(a, dtype=np.float32).astype(ml_dtypes.bfloat16)


def _pcols(vec, ncols):
    """[ncols*128] -> [128, ncols], col j = vec[j*128:(j+1)*128]."""
    return np.ascontiguousarray(
        np.asarray(vec, dtype=np.float32).reshape(ncols, 128).T)


def prepare_inputs(inputs):
    X = np.asarray(inputs["X"], dtype=np.float32)
    S = np.asarray(inputs["S_state"], dtype=np.float32)
    bias = np.asarray(inputs["bias"], dtype=np.float32)
    tok2grid = np.asarray(inputs["tok2grid"]).astype(np.int64)
    Wb = np.asarray(inputs["Wb"], dtype=np.float32)
    Wq = np.asarray(inputs["Wq"], dtype=np.float32)
    Wk = np.asarray(inputs["Wk"], dtype=np.float32)
    Wv = np.asarray(inputs["Wv"], dtype=np.float32)
    Wo_ = np.asarray(inputs["Wo"], dtype=np.float32)
    fc1_W = np.asarray(inputs["fc1_W"], dtype=np.float32)
    fc2_W = np.asarray(inputs["fc2_W"], dtype=np.float32)
    conv_W = np.asarray(inputs["conv_W"], dtype=np.float32)
    bq = np.asarray(inputs["bq"], dtype=np.float32)
    bv = np.asarray(inputs["bv"], dtype=np.float32)

    shared = {}
    # block-diagonal Wb over m-chunks of 8: [(d,mi), (h,mj)] per layer
    wblk = np.zeros([NL, 8, DB, H, 8], np.float32)
    for mi in range(8):
        wblk[:, mi, :, :, mi] = Wb
    shared["wbblk"] = _bf(np.ascontiguousarray(
        wblk.transpose(0, 2, 1, 3, 4).reshape(NL, 128, 64)
        .transpose(1, 0, 2)))
    shared["wqkv"] = _bf(np.concatenate(
        [Wq.transpose(0, 2, 1, 3).reshape(NL, DD, H * DA),
         Wk.transpose(0, 2, 1, 3).reshape(NL, DD, H * DA),
         Wv.transpose(0, 2, 1, 3).reshape(NL, DD, H * DM)], axis=2))
    shared["wo"] = _bf(Wo_)
    shared["fc1w"] = _bf(fc1_W)
    shared["fc2w"] = _bf(fc2_W)
    cw = conv_W.transpose(0, 2, 3, 4, 1).reshape(LC, DD, 9 * DD)
    shared["cww"] = _bf(cw.reshape(LC, 2, 128, 9 * DD))

    lng = np.stack([
        *[_pcols(inputs["ln1_g"][l], 2) for l in range(NL)],
        *[_pcols(inputs["ln2_g"][l], 2) for l in range(NL)],
        *[_pcols(inputs["cln_g"][c], 2) for c in range(LC)]])
    lnbv = np.stack([
        *[_pcols(inputs["ln1_b"][l], 2) for l in range(NL)],
        *[_pcols(inputs["ln2_b"][l], 2) for l in range(NL)],
        *[_pcols(inputs["cln_b"][c], 2) for c in range(LC)]])
    order = [0, 3, 1, 4, 2, 5, 6, 7, 8]
    shared["lngP"] = np.ascontiguousarray(lng[order].transpose(1, 0, 2))
    shared["lnbP"] = np.ascontiguousarray(lnbv[order].transpose(1, 0, 2))
    shared["boP"] = np.ascontiguousarray(np.stack(
        [_pcols(inputs["bo"][l], 2) for l in range(NL)], axis=1))
    shared["b1P"] = np.ascontiguousarray(np.stack(
        [_pcols(inputs["fc1_b"][l], 8) for l in range(NL)], axis=1))
    shared["b2P"] = np.ascontiguousarray(np.stack(
        [_pcols(inputs["fc2_b"][l], 2) for l in range(NL)], axis=1))
    shared["cbP"] = np.ascontiguousarray(np.stack(
        [_pcols(inputs["conv_b"][c], 2) for c in range(NL)], axis=1))
    shared["pP"] = _pcols(inputs["P"], 2)
    bqf = bq.reshape(NL, H * DA)
    shared["bqP"] = np.ascontiguousarray(np.stack(
        [_pcols(bqf[l], 4) for l in range(NL)], axis=1))
    bvf = bv.reshape(NL, H * DM)
    shared["bvP"] = _bf(np.ascontiguousarray(np.stack(
        [_pcols(bvf[l], 4) for l in range(NL)], axis=1)))
    shared["eyeD"] = _bf(np.eye(128, dtype=np.float32))
    obt = np.zeros([16, 16, 64], np.float32)
    for b in range(16):
        obt[b, b, :] = 1.0
    shared["obtD"] = _bf(obt.reshape(16, 1024))

    in_maps = []
    for c in range(NCORES):
        bs = slice(c * BL, (c + 1) * BL)
        m = dict(shared)
        rs2 = 1.0 / math.sqrt(2.0)
        m["xT"] = np.ascontiguousarray(
            X[bs].transpose(2, 0, 1).reshape(DD, R)) * rs2
        m["sT"] = np.ascontiguousarray(
            S[bs].transpose(2, 0, 1).reshape(DD, R)) * rs2
        # [mc, (d,mi), (b,l)]: biasT2[mc, d*8+mi, b*L+l] = bias[b,l,8mc+mi,d]
        bt2 = bias[bs].transpose(2, 3, 0, 1)          # [m, d, b, l]
        bt2 = bt2.reshape(MC, 8, DB, BL, L)           # [mc, mi, d, b, l]
        bt2 = bt2.transpose(0, 2, 1, 3, 4).reshape(MC, 128, BL * L)
        m["biasT2"] = _bf(np.ascontiguousarray(bt2))
        in_maps.append(m)
    return in_maps, tok2grid


_GRAPH_CACHE = {}


def kernel(**inputs):
    in_maps, tok2grid = prepare_inputs(inputs)
    key = tuple(int(v) for v in tok2grid)
    if key not in _GRAPH_CACHE:
        _GRAPH_CACHE[key] = build_graph(tok2grid)[0]
    nc = _GRAPH_CACHE[key]
    res = run_bass_kernel_spmd(nc, in_maps, core_ids=list(range(NCORES)))
    outs = []
    for c in range(NCORES):
        o = res.results[c]["out"].reshape(DD, BL, L).transpose(1, 2, 0)
        outs.append(o)
    return np.ascontiguousarray(np.concatenate(outs, axis=0),
                                dtype=np.float32)
